# revision 18
# baseline (speedup 1.0000x reference)
# Trainium2 Bass kernel for a pre-norm transformer block with forward-mode JVP
# (jax.linearize) over M=4 tangent directions.
#
# Sharding: 8 cores; core c handles batch b=c//2 and tangents {2*(c%2), 2*(c%2)+1}.
# Each core computes the primal pass for its batch (even/odd core pairs do this
# redundantly; the even core's primal is used) plus 2 tangent JVP passes.
# No cross-core communication.
#
# On-chip layout is feature-major ([D, S] with features on partitions), so every
# linear layer is a plain accumulated matmul with no transposes. LayerNorm /
# softmax statistics (which reduce over partitions in this layout) are computed
# on the PE via ones-matmuls whose M=128 output broadcasts the column sums to
# all partitions. Softmax is computed without max-subtraction (scores are O(3)
# for this problem's data distribution). LN affine (g, b) is folded into the
# weights on the host; biases enter via K=1 ones-row matmuls; the final mlp
# bias bf2 is added on the host.
#
# Matmul inputs are bf16 (fp32 PSUM accumulation); the residual stream and LN /
# softmax statistics stay fp32 (residual adds read the fp32 inputs re-DMAed
# from DRAM; xa is spilled to DRAM scratch between the attention and MLP
# residual adds to stay under the SBUF budget).

import numpy as np
import ml_dtypes

import concourse.bass as bass
import concourse.tile as tile
from concourse import bacc, mybir
from concourse.bass_utils import run_bass_kernel_spmd

AF = mybir.ActivationFunctionType
BF = mybir.dt.bfloat16
F32 = mybir.dt.float32

B, S, D, H, M = 4, 512, 768, 12, 4
DFF = 4 * D
HD = D // H
EPS = 1e-6
SCALE = HD ** -0.5
NK = D // 128          # 6 feature chunks
NSC = S // 128         # 4 sequence chunks
NM1 = DFF // 128       # 24
N_CORES = 8

_CACHE = {}
_RUN_KWARGS = {}   # test harness can set {"trace": True}
_LAST_RES = [None]


def _build_program(with_bias):
    nc = bacc.Bacc("TRN2", target_bir_lowering=False, debug=False,
                   num_devices=N_CORES)

    dram = {}
    def din(name, shape, dt):
        dram[name] = nc.dram_tensor(name, shape, dt, kind="ExternalInput").ap()
    din("x_f32", [NK, 128, S], F32)
    din("x_bf", [NK, 128, S], BF)
    din("t_f32", [2, NK, 128, S], F32)
    din("t_bf", [2, NK, 128, S], BF)
    din("wqkv", [NK, 128, 3 * D], BF)
    din("wproj", [NK, 128, D], BF)
    din("w1", [NK, 128, DFF], BF)
    din("w2", [NM1, 128, D], BF)
    din("bqk", [1, 2 * D], BF)
    din("bv", [1, D], BF)
    din("bproj", [1, D], BF)
    din("b1m", [1, DFF], BF)
    out_d = nc.dram_tensor("out", [3, NK, 128, S], F32, kind="ExternalOutput").ap()
    xa_scr = nc.dram_tensor("xa_scr", [3, NK, 128, S], F32).ap()
    xab_scr = nc.dram_tensor("xab_scr", [3, NK, 128, S], BF).ap()

    with tile.TileContext(nc) as tc:
        _emit(nc, tc, dram, out_d, xa_scr, xab_scr, with_bias)
    nc.compile()
    return nc


def _emit(nc, tc, dram, out_d, xa_scr, xab_scr, with_bias):
    from contextlib import ExitStack
    ctx = ExitStack()
    with ctx:
        pool_const = ctx.enter_context(tc.tile_pool(name="const", bufs=1))
        pool_rot = ctx.enter_context(tc.tile_pool(name="rot", bufs=2))
        pool_stat = ctx.enter_context(tc.tile_pool(name="stat", bufs=1))
        ps_big = ctx.enter_context(tc.tile_pool(name="psbig", bufs=5, space="PSUM"))
        ps_st = ctx.enter_context(tc.tile_pool(name="psst", bufs=3, space="PSUM"))

        # ---- constants ----
        ones1 = pool_const.tile([128, 128], BF, tag="ones1")
        nc.gpsimd.memset(ones1[:], 1.0)
        onesd = pool_const.tile([128, 128], BF, tag="onesd")
        nc.gpsimd.memset(onesd[:], 1.0 / D)
        onesrow = pool_const.tile([1, S], BF, tag="onesrow")
        nc.gpsimd.memset(onesrow[:], 1.0)
        epsb = pool_const.tile([128, 1], F32, tag="epsb")
        nc.gpsimd.memset(epsb[:], EPS)
        bqk_sb = pool_const.tile([1, 2 * D], BF, tag="bqk")
        nc.sync.dma_start(bqk_sb[:], dram["bqk"][:])
        bv_sb = pool_const.tile([1, D], BF, tag="bv")
        nc.sync.dma_start(bv_sb[:], dram["bv"][:])
        bproj_sb = pool_const.tile([1, D], BF, tag="bproj")
        nc.sync.dma_start(bproj_sb[:], dram["bproj"][:])
        b1m_sb = pool_const.tile([1, DFF], BF, tag="b1m")
        nc.sync.dma_start(b1m_sb[:], dram["b1m"][:])

        def ck(t, k):
            return t[:, k * S:(k + 1) * S]

        def f32tmp():
            return pool_rot.tile([128, S], F32, tag="f32tmp", bufs=3,
                                 name="f32tmp")

        # ---- LayerNorm: primal part (stats via PE colsum-broadcast) ----
        def ln_primal(in_bf, n_bf):
            mu_ps = ps_st.tile([128, S], F32, tag="st", name="mu_ps")
            s2_ps = ps_st.tile([128, S], F32, tag="st", name="s2_ps")
            for k in range(NK):
                sq = pool_rot.tile([128, S], BF, tag="sq", name="sq")
                nc.vector.tensor_mul(sq[:], ck(in_bf, k), ck(in_bf, k))
                nc.tensor.matmul(mu_ps[:], onesd[:], ck(in_bf, k),
                                 start=(k == 0), stop=(k == NK - 1))
                nc.tensor.matmul(s2_ps[:], onesd[:], sq[:],
                                 start=(k == 0), stop=(k == NK - 1))
            mu_f = pool_stat.tile([128, S], F32, tag="lnmu", name="mu_f")
            nc.scalar.copy(mu_f[:], mu_ps[:])
            mu2 = pool_rot.tile([128, S], BF, tag="sq", name="mu2")
            nc.vector.tensor_mul(mu2[:], mu_f[:], mu_f[:])
            var = f32tmp()
            nc.vector.tensor_sub(var[:], s2_ps[:], mu2[:])
            sd = f32tmp()
            nc.scalar.activation(sd[:], var[:], AF.Sqrt, bias=epsb[:])
            r_f = pool_stat.tile([128, S], F32, tag="lnr", name="r_f")
            nc.vector.reciprocal_approx_fast(r_f[:], sd[:])
            for k in range(NK):
                cen = pool_rot.tile([128, S], F32, tag="cen", name="cen")
                nc.vector.tensor_sub(cen[:], ck(in_bf, k), mu_f[:])
                nc.vector.tensor_mul(ck(n_bf, k), cen[:], r_f[:])
            return r_f

        # ---- LayerNorm: one tangent's JVP ----
        def ln_tangent(in_bf, tan_bf, n_bf, r_bf, dn_bf):  # r_bf is f32 now
            mt_ps = ps_st.tile([128, S], F32, tag="st", name="mt_ps")
            c_ps = ps_st.tile([128, S], F32, tag="st", name="c_ps")
            for k in range(NK):
                p = pool_rot.tile([128, S], BF, tag="p", name="p")
                nc.vector.tensor_mul(p[:], ck(n_bf, k), ck(tan_bf, k))
                nc.tensor.matmul(mt_ps[:], onesd[:], ck(tan_bf, k),
                                 start=(k == 0), stop=(k == NK - 1))
                nc.tensor.matmul(c_ps[:], onesd[:], p[:],
                                 start=(k == 0), stop=(k == NK - 1))
            ctr = pool_rot.tile([128, S], F32, tag="ctr", name="ctr")
            nc.vector.tensor_mul(ctr[:], c_ps[:], r_bf[:])
            mt_f = pool_rot.tile([128, S], F32, tag="ctr", name="mt_f")
            nc.scalar.copy(mt_f[:], mt_ps[:])
            for k in range(NK):
                b_ = pool_rot.tile([128, S], F32, tag="cen", name="b_")
                nc.vector.tensor_sub(b_[:], ck(tan_bf, k), mt_f[:])
                e_ = pool_rot.tile([128, S], F32, tag="e", name="e_")
                nc.vector.tensor_mul(e_[:], b_[:], r_bf[:])
                f_ = pool_rot.tile([128, S], F32, tag="f", name="f_")
                nc.vector.tensor_mul(f_[:], ck(n_bf, k), ctr[:])
                nc.vector.tensor_sub(ck(dn_bf, k), e_[:], f_[:])

        # LN input/output pools outlive the o_sb pool (LIFO nesting); the
        # LN2 tensors reuse the LN1 tags (slot reuse after LN1 consumers end).
        st_n2 = ExitStack()
        pool_lnout = st_n2.enter_context(tc.tile_pool(name="lnout", bufs=1))
        st_ab = ExitStack()
        pool_lnio = st_ab.enter_context(tc.tile_pool(name="lnio", bufs=1))

        st_o = ExitStack()
        pool_o = st_o.enter_context(tc.tile_pool(name="osb", bufs=1))
        o_sb = pool_o.tile([128, NK * S], BF, tag="o", name="o_sb")
        do_sb = [pool_o.tile([128, NK * S], BF, tag=f"do{m}", name=f"do_sb{m}")
                 for m in range(2)]

        st_qkv = ExitStack()
        pool_qkv = st_qkv.enter_context(tc.tile_pool(name="qkv", bufs=1))
        q_sb = pool_qkv.tile([128, NK * S], BF, tag="q", name="q_sb")
        k_sb = pool_qkv.tile([128, NK * S], BF, tag="k", name="k_sb")
        dq_sb = [pool_qkv.tile([128, NK * S], BF, tag=f"dq{m}", name=f"dq_sb{m}")
                 for m in range(2)]
        dk_sb = [pool_qkv.tile([128, NK * S], BF, tag=f"dk{m}", name=f"dk_sb{m}")
                 for m in range(2)]
        v_sb = pool_qkv.tile([128, NSC * D], BF, tag="v", name="v_sb")
        dv_sb = [pool_qkv.tile([128, NSC * D], BF, tag=f"dv{m}", name=f"dv_sb{m}")
                 for m in range(2)]

        # =========== Phases A+B: LN1 and QKV, primal first ===========
        with tc.tile_pool(name="wq", bufs=1) as pool_wq:
            n1 = pool_lnout.tile([128, NK * S], BF, tag="n1", name="n1")
            dn1 = [pool_lnout.tile([128, NK * S], BF, tag=f"dn1{m}",
                                   name=f"dn1{m}") for m in range(2)]
            xbf = pool_lnio.tile([128, NK * S], BF, tag="xbf", name="xbf")
            tbf = [pool_lnio.tile([128, NK * S], BF, tag=f"tbf{m}",
                                  name=f"tbf{m}") for m in range(2)]
            for k in range(NK):
                nc.scalar.dma_start(ck(xbf, k), dram["x_bf"][k])
                for m in range(2):
                    nc.scalar.dma_start(ck(tbf[m], k), dram["t_bf"][m, k])
            wqkv_t = []
            for k in range(NK):
                wt = pool_wq.tile([128, 3 * D], BF, tag=f"wqkv{k}",
                                  name=f"wqkv{k}")
                nc.sync.dma_start(wt[:], dram["wqkv"][k])
                wqkv_t.append(wt)

            r1_bf = ln_primal(xbf, n1)

            def qk_pass2(src, qd, kd, with_bias):
                for mt in range(12):
                    ps = ps_big.tile([128, S], F32, tag="big", name="qkv_ps")
                    for k in range(NK):
                        nc.tensor.matmul(ps[:],
                                         wqkv_t[k][:, mt * 128:(mt + 1) * 128],
                                         ck(src, k), start=(k == 0),
                                         stop=(k == NK - 1 and not with_bias))
                    if with_bias:
                        nc.tensor.matmul(ps[:],
                                         bqk_sb[0:1, mt * 128:(mt + 1) * 128],
                                         onesrow[:], start=False, stop=True)
                    if mt < 6:
                        nc.scalar.mul(qd[:, mt * S:(mt + 1) * S], ps[:], SCALE)
                    else:
                        nc.scalar.copy(kd[:, (mt - 6) * S:(mt - 5) * S], ps[:])

            def v_pass(src, vd, with_bias):
                for sc in range(NSC):
                    for g2 in range(2):
                        wv_col = 2 * D + g2 * 384
                        ps = ps_big.tile([128, 384], F32, tag="big", name="v_ps")
                        for k in range(NK):
                            lhs = src[:, k * S + sc * 128: k * S + (sc + 1) * 128]
                            nc.tensor.matmul(ps[:], lhs,
                                             wqkv_t[k][:, wv_col:wv_col + 384],
                                             start=(k == 0),
                                             stop=(k == NK - 1 and not with_bias))
                        if with_bias:
                            nc.tensor.matmul(ps[:], ones1[0:1, :],
                                             bv_sb[0:1, g2 * 384:(g2 + 1) * 384],
                                             start=False, stop=True)
                        col = sc * D + g2 * 384
                        nc.scalar.copy(vd[:, col:col + 384], ps[:])

            qk_pass2(n1, q_sb, k_sb, with_bias)
            v_pass(n1, v_sb, with_bias)
            for m in range(2):
                ln_tangent(xbf, tbf[m], n1, r1_bf, dn1[m])
            for m in range(2):
                qk_pass2(dn1[m], dq_sb[m], dk_sb[m], False)
                v_pass(dn1[m], dv_sb[m], False)

        # =========== Phase C: attention, head pairs ===========
        # Heads are processed in pairs (2i, 2i+1) sharing one feature chunk:
        # the K=64 score matmuls of the two heads run concurrently in the PE
        # via row tile_position (0,0)/(64,0); the M=64 attention-value matmuls
        # share one PSUM bank via column tile_position (0,0)/(0,64).
        # Two-stage software pipeline as before, one pair per iteration.
        with tc.tile_pool(name="head2", bufs=2) as pool_h2, \
             tc.tile_pool(name="head1", bufs=1) as pool_h1:
            def stage1(pi):
                kc = pi
                qp = q_sb[:, kc * S:(kc + 1) * S]
                kp = k_sb[:, kc * S:(kc + 1) * S]
                sT = {}
                expT = [pool_h2.tile([128, NSC * S], BF, tag=f"exp{j}",
                                     name=f"expT{j}") for j in range(2)]
                for c in range(NSC):
                    for j, po in ((0, 0), (1, 64)):
                        st = ps_big.tile([128, S], F32, tag="big", name="sT")
                        nc.tensor.matmul(st[:],
                                         kp[po:po + 64, c * 128:(c + 1) * 128],
                                         qp[po:po + 64, :], start=True,
                                         stop=True, tile_position=(po, 0))
                        sT[(j, c)] = st
                    for j in range(2):
                        nc.scalar.activation(ck(expT[j], c), sT[(j, c)][:],
                                             AF.Exp)
                ds_sb = []
                for m in range(2):
                    dqp = dq_sb[m][:, kc * S:(kc + 1) * S]
                    dkp = dk_sb[m][:, kc * S:(kc + 1) * S]
                    dsb = [pool_h2.tile([128, NSC * S], BF, tag=f"ds{m}{j}",
                                        name=f"ds_sb{m}{j}", bufs=1)
                           for j in range(2)]
                    for c in range(NSC):
                        dsp = {}
                        for j, po in ((0, 0), (1, 64)):
                            ps = ps_big.tile([128, S], F32, tag="big",
                                             name="dsT")
                            nc.tensor.matmul(
                                ps[:], dkp[po:po + 64, c * 128:(c + 1) * 128],
                                qp[po:po + 64, :], start=True, stop=False,
                                tile_position=(po, 0))
                            nc.tensor.matmul(
                                ps[:], kp[po:po + 64, c * 128:(c + 1) * 128],
                                dqp[po:po + 64, :], start=False, stop=True,
                                tile_position=(po, 0))
                            dsp[j] = ps
                        for j in range(2):
                            if m == 0:
                                nc.scalar.copy(ck(dsb[j], c), dsp[j][:])
                            else:
                                nc.vector.tensor_copy(ck(dsb[j], c), dsp[j][:])
                    ds_sb.append(dsb)
                return expT, ds_sb

            def stage2(pi, expT, ds_sb):
                kc = pi
                sums = [ps_st.tile([128, S], F32, tag="st", name="sums")
                        for _ in range(2)]
                for j in range(2):
                    for c in range(NSC):
                        nc.tensor.matmul(sums[j][:], ones1[:], ck(expT[j], c),
                                         start=(c == 0), stop=(c == NSC - 1))
                rinv = []
                for j in range(2):
                    rf = f32tmp()
                    nc.vector.reciprocal_approx_fast(rf[:], sums[j][:])
                    rv = pool_h1.tile([128, S], BF, tag=f"rinv{j}",
                                      name=f"rinv{j}")
                    nc.scalar.copy(rv[:], rf[:])
                    rinv.append(rv)
                at = [pool_h1.tile([128, NSC * S], BF, tag=f"at{j}",
                                   name=f"at{j}") for j in range(2)]
                for j in range(2):
                    for c in range(NSC):
                        nc.vector.tensor_mul(ck(at[j], c), ck(expT[j], c),
                                             rinv[j][:])
                o_ps = ps_st.tile([128, S], F32, tag="st", name="o_ps")
                for c in range(NSC):
                    for j, po in ((0, 0), (1, 64)):
                        h = 2 * pi + j
                        vh = v_sb[:, c * D + h * HD: c * D + (h + 1) * HD]
                        nc.tensor.matmul(o_ps[po:po + 64, :], vh, ck(at[j], c),
                                         start=(c == 0), stop=(c == NSC - 1),
                                         tile_position=(0, po))
                osl = o_sb[:, kc * S:(kc + 1) * S]
                nc.scalar.copy(osl, o_ps[:])
                for m in range(2):
                    pt = [pool_h1.tile([128, NSC * S], BF, tag=f"pt{j}",
                                       name=f"pt{j}") for j in range(2)]
                    for j in range(2):
                        for c in range(NSC):
                            nc.vector.tensor_mul(ck(pt[j], c), ck(at[j], c),
                                                 ck(ds_sb[m][j], c))
                    c_ps = [ps_st.tile([128, S], F32, tag="st", name="c_ps2")
                            for _ in range(2)]
                    for j in range(2):
                        for c in range(NSC):
                            nc.tensor.matmul(c_ps[j][:], ones1[:],
                                             ck(pt[j], c), start=(c == 0),
                                             stop=(c == NSC - 1))
                    do_ps = ps_st.tile([128, S], F32, tag="st", name="do_ps")
                    for c in range(NSC):
                        for j, po in ((0, 0), (1, 64)):
                            h = 2 * pi + j
                            vh = v_sb[:, c * D + h * HD: c * D + (h + 1) * HD]
                            dvh = dv_sb[m][:, c * D + h * HD:
                                           c * D + (h + 1) * HD]
                            nc.tensor.matmul(do_ps[po:po + 64, :], vh,
                                             ck(pt[j], c), start=(c == 0),
                                             stop=False, tile_position=(0, po))
                            nc.tensor.matmul(do_ps[po:po + 64, :], dvh,
                                             ck(at[j], c), start=False,
                                             stop=(c == NSC - 1),
                                             tile_position=(0, po))
                    corr = pool_rot.tile([128, S], F32, tag="corr", name="corr", bufs=1)
                    for j, po in ((0, 0), (1, 64)):
                        nc.vector.tensor_mul(corr[po:po + 64, :],
                                             o_sb[po:po + 64,
                                                  kc * S:(kc + 1) * S],
                                             c_ps[j][po:po + 64, :])
                    nc.vector.tensor_sub(do_sb[m][:, kc * S:(kc + 1) * S],
                                         do_ps[:], corr[:])

            pend = None
            for pi in range(H // 2):
                args = stage1(pi)
                if pend is not None:
                    stage2(*pend)
                pend = (pi, *args)
            stage2(*pend)
        st_qkv.close()

        # =========== Phase D: proj + residual (spill xa f32 to DRAM) ========
        with tc.tile_pool(name="wp", bufs=1) as pool_wp, \
             tc.tile_pool(name="resid", bufs=1) as pool_res:
            # warm the sqrt table set while the PE is still busy, so LN2's
            # critical path doesn't pay the ACT table load
            warm = pool_rot.tile([128, 1], F32, tag="warm", name="warm")
            nc.scalar.activation(warm[:], epsb[:], AF.Sqrt, bias=epsb[:])
            wproj_t = []
            for k in range(NK):
                wt = pool_wp.tile([128, D], BF, tag=f"wp{k}", name=f"wp{k}")
                nc.sync.dma_start(wt[:], dram["wproj"][k])
                wproj_t.append(wt)
            # prefetch all residual inputs
            res_in = pool_res.tile([128, 3 * NK * S], F32, tag="resin",
                                   name="res_in")
            for si in range(3):
                for k in range(NK):
                    dst = res_in[:, (si * NK + k) * S:(si * NK + k + 1) * S]
                    if si == 0:
                        nc.scalar.dma_start(dst, dram["x_f32"][k])
                    else:
                        nc.scalar.dma_start(dst, dram["t_f32"][si - 1, k])
            xabf = pool_lnio.tile([128, NK * S], BF, tag="xbf", name="xabf")
            dxabf = [pool_lnio.tile([128, NK * S], BF, tag=f"tbf{m}",
                                    name=f"dxabf{m}") for m in range(2)]
            psrcs = [o_sb, do_sb[0], do_sb[1]]
            pdsts = [xabf, dxabf[0], dxabf[1]]
            for mt in range(NK):
                for si in range(3):
                    ps = ps_big.tile([128, S], F32, tag="big", name="pj_ps")
                    for k in range(NK):
                        nc.tensor.matmul(ps[:],
                                         wproj_t[k][:, mt * 128:(mt + 1) * 128],
                                         ck(psrcs[si], k), start=(k == 0),
                                         stop=(k == NK - 1 and
                                               (si != 0 or not with_bias)))
                    if si == 0 and with_bias:
                        nc.tensor.matmul(
                            ps[:], bproj_sb[0:1, mt * 128:(mt + 1) * 128],
                            onesrow[:], start=False, stop=True)
                    xaf = f32tmp()
                    nc.vector.tensor_add(
                        xaf[:], res_in[:, (si * NK + mt) * S:
                                       (si * NK + mt + 1) * S], ps[:])
                    nc.sync.dma_start(xa_scr[si, mt], xaf[:])
                    nc.scalar.copy(ck(pdsts[si], mt), xaf[:])
        st_o.close()

        # =========== Phase E: LN2 (primal first) ===========
        n2 = pool_lnout.tile([128, NK * S], BF, tag="n1", name="n2")
        dn2 = [pool_lnout.tile([128, NK * S], BF, tag=f"dn1{m}", name=f"dn2{m}")
               for m in range(2)]
        r2_bf = ln_primal(xabf, n2)
        for m in range(2):
            ln_tangent(xabf, dxabf[m], n2, r2_bf, dn2[m])

        # =========== Phase F1: W1 + gelu, primal first ===========
        st_g = ExitStack()
        pool_g = st_g.enter_context(tc.tile_pool(name="gq", bufs=1))
        g_sb = pool_g.tile([128, NM1 * S], BF, tag="g", name="g_sb")
        q2_sb = [pool_g.tile([128, NM1 * S], BF, tag=f"q2{m}", name=f"q2_sb{m}")
                 for m in range(2)]
        with tc.tile_pool(name="dgp", bufs=1) as pool_dg:
            for half in range(2):
                dg_sb = pool_dg.tile([128, (NM1 // 2) * S], BF, tag="dg",
                                     name="dg_sb")
                with tc.tile_pool(name=f"w1h{half}", bufs=1) as pool_w1:
                    w1_t = []
                    for k in range(NK):
                        wt = pool_w1.tile([128, DFF // 2], BF, tag=f"w1{k}",
                                          name=f"w1_{half}_{k}")
                        nc.sync.dma_start(
                            wt[:], dram["w1"][k][:, half * (DFF // 2):
                                                 (half + 1) * (DFF // 2)])
                        w1_t.append(wt)
                    for mi in range(NM1 // 2):
                        mt = half * (NM1 // 2) + mi
                        ps = ps_big.tile([128, S], F32, tag="big", name="u_ps")
                        for k in range(NK):
                            nc.tensor.matmul(ps[:],
                                             w1_t[k][:, mi * 128:(mi + 1) * 128],
                                             ck(n2, k), start=(k == 0),
                                             stop=(k == NK - 1 and
                                                   not with_bias))
                        if with_bias:
                            nc.tensor.matmul(
                                ps[:], b1m_sb[0:1, mt * 128:(mt + 1) * 128],
                                onesrow[:], start=False, stop=True)
                        nc.scalar.activation(ck(g_sb, mt), ps[:], AF.Gelu)
                        nc.scalar.activation(ck(dg_sb, mi), ps[:],
                                             AF.Derivative_Gelu)
                    for m in range(2):
                        for mi in range(NM1 // 2):
                            mt = half * (NM1 // 2) + mi
                            ps = ps_big.tile([128, S], F32, tag="big",
                                             name="ut_ps")
                            for k in range(NK):
                                nc.tensor.matmul(
                                    ps[:], w1_t[k][:, mi * 128:(mi + 1) * 128],
                                    ck(dn2[m], k), start=(k == 0),
                                    stop=(k == NK - 1))
                            nc.vector.tensor_mul(ck(q2_sb[m], mt),
                                                 ck(dg_sb, mi), ps[:])

        # =========== Phase F2: W2 + final residual ===========
        fsrcs = [g_sb, q2_sb[0], q2_sb[1]]
        with tc.tile_pool(name="w2p", bufs=1) as pool_w2, \
             tc.tile_pool(name="res2", bufs=1) as pool_res2:
            w2_t = []
            for k in range(NM1):
                wt = pool_w2.tile([128, D], BF, tag=f"w2{k}", name=f"w2_{k}")
                nc.scalar.dma_start(wt[:], dram["w2"][k])
                w2_t.append(wt)
            for si in range(3):
                for mt in range(NK):
                    res2 = pool_res2.tile([128, S], F32, tag="res2", bufs=6,
                                          name="res2")
                    nc.sync.dma_start(res2[:], xa_scr[si, mt])
                    ps = ps_big.tile([128, S], F32, tag="big", name="o2_ps")
                    for k in range(NM1):
                        nc.tensor.matmul(ps[:],
                                         w2_t[k][:, mt * 128:(mt + 1) * 128],
                                         ck(fsrcs[si], k), start=(k == 0),
                                         stop=(k == NM1 - 1))
                    ot = f32tmp()
                    nc.vector.tensor_add(ot[:], res2[:], ps[:])
                    nc.sync.dma_start(out_d[si, mt], ot[:])
        st_g.close()
        st_ab.close()
        st_n2.close()


def _prep_host(inputs):
    f32 = np.float32
    x = np.asarray(inputs["x"], f32)
    xt = np.asarray(inputs["x_tangent"], f32)
    g1 = np.asarray(inputs["g1"], f32); b1 = np.asarray(inputs["b1"], f32)
    g2 = np.asarray(inputs["g2"], f32); b2 = np.asarray(inputs["b2"], f32)
    Wqkv = np.asarray(inputs["Wqkv"], f32); Wproj = np.asarray(inputs["Wproj"], f32)
    W1 = np.asarray(inputs["W1"], f32); W2 = np.asarray(inputs["W2"], f32)
    bproj = np.asarray(inputs["bproj"], f32)
    bf1 = np.asarray(inputs["bf1"], f32); bf2 = np.asarray(inputs["bf2"], f32)

    bf16 = ml_dtypes.bfloat16
    Wqkv_f = g1[:, None] * Wqkv
    bqkv = b1 @ Wqkv
    W1_f = g2[:, None] * W1
    b1m = b2 @ W1 + bf1

    def tile_k(w, nk):
        return np.ascontiguousarray(w.reshape(nk, 128, -1))

    shared = {
        "wqkv": tile_k(Wqkv_f, NK).astype(bf16),
        "wproj": tile_k(Wproj, NK).astype(bf16),
        "w1": tile_k(W1_f, NK).astype(bf16),
        "w2": tile_k(W2, NM1).astype(bf16),
        "bqk": np.ascontiguousarray(bqkv[None, :2 * D]).astype(bf16),
        "bv": np.ascontiguousarray(bqkv[None, 2 * D:]).astype(bf16),
        "bproj": np.ascontiguousarray(bproj[None, :]).astype(bf16),
        "b1m": np.ascontiguousarray(b1m[None, :]).astype(bf16),
    }
    in_maps = []
    for core in range(N_CORES):
        b, mp = core // 2, core % 2
        xT = np.ascontiguousarray(x[b].T).reshape(NK, 128, S)
        tT = np.ascontiguousarray(
            xt[b, 2 * mp:2 * mp + 2].transpose(0, 2, 1)).reshape(2, NK, 128, S)
        im = dict(shared)
        im["x_f32"] = xT
        im["x_bf"] = xT.astype(bf16)
        im["t_f32"] = tT
        im["t_bf"] = tT.astype(bf16)
        in_maps.append(im)
    return in_maps, bf2


def kernel(**inputs):
    with_bias = not all(
        np.allclose(np.asarray(inputs[k]), 0.0)
        for k in ("b1", "bproj", "b2", "bf1"))
    key = ("nc", with_bias)
    if key not in _CACHE:
        _CACHE[key] = _build_program(with_bias)
    nc = _CACHE[key]
    in_maps, bf2 = _prep_host(inputs)
    res = run_bass_kernel_spmd(nc, in_maps, core_ids=list(range(N_CORES)),
                               **_RUN_KWARGS)
    _LAST_RES[0] = res
    out = np.zeros((B, S, D), np.float32)
    out_tan = np.zeros((B, M, S, D), np.float32)
    for core in range(N_CORES):
        b, mp = core // 2, core % 2
        o = res.results[core]["out"].reshape(3, D, S)
        if mp == 0:
            out[b] = o[0].T + bf2[None, :]
        out_tan[b, 2 * mp] = o[1].T
        out_tan[b, 2 * mp + 1] = o[2].T
    return out, out_tan


# revision 19
# speedup vs baseline: 1.1994x; 1.1994x over previous
# Trainium2 Bass kernel for a pre-norm transformer block with forward-mode JVP
# (jax.linearize) over M=4 tangent directions.
#
# Sharding: 8 cores; core c handles batch b=c//2 and tangents {2*(c%2), 2*(c%2)+1}.
# Each core computes the primal pass for its batch (even/odd core pairs do this
# redundantly; the even core's primal is used) plus 2 tangent JVP passes.
# No cross-core communication.
#
# On-chip layout is feature-major ([D, S] with features on partitions), so every
# linear layer is a plain accumulated matmul with no transposes. LayerNorm /
# softmax statistics (which reduce over partitions in this layout) are computed
# on the PE via ones-matmuls whose M=128 output broadcasts the column sums to
# all partitions. Softmax is computed without max-subtraction (scores are O(3)
# for this problem's data distribution). LN affine (g, b) is folded into the
# weights on the host; biases enter via K=1 ones-row matmuls; the final mlp
# bias bf2 is added on the host.
#
# Matmul inputs are bf16 (fp32 PSUM accumulation); the residual stream and LN /
# softmax statistics stay fp32 (residual adds read the fp32 inputs re-DMAed
# from DRAM; xa is spilled to DRAM scratch between the attention and MLP
# residual adds to stay under the SBUF budget).

import numpy as np
import ml_dtypes

import concourse.bass as bass
import concourse.tile as tile
from concourse import bacc, mybir
from concourse.bass_utils import run_bass_kernel_spmd

AF = mybir.ActivationFunctionType
BF = mybir.dt.bfloat16
F32 = mybir.dt.float32

B, S, D, H, M = 4, 512, 768, 12, 4
DFF = 4 * D
HD = D // H
EPS = 1e-6
SCALE = HD ** -0.5
NK = D // 128          # 6 feature chunks
NSC = S // 128         # 4 sequence chunks
NM1 = DFF // 128       # 24
N_CORES = 8

_CACHE = {}
_RUN_KWARGS = {}   # test harness can set {"trace": True}
_LAST_RES = [None]


def _build_program(with_bias):
    nc = bacc.Bacc("TRN2", target_bir_lowering=False, debug=False,
                   num_devices=N_CORES)

    dram = {}
    def din(name, shape, dt):
        dram[name] = nc.dram_tensor(name, shape, dt, kind="ExternalInput").ap()
    din("x_f32", [NK, 128, S], F32)
    din("x_bf", [NK, 128, S], BF)
    din("t_f32", [2, NK, 128, S], F32)
    din("t_bf", [2, NK, 128, S], BF)
    din("wqkv", [NK, 128, 3 * D], BF)
    din("wproj", [NK, 128, D], BF)
    din("w1", [NK, 128, DFF], BF)
    din("w2", [NM1, 128, D], BF)
    din("bqk", [1, 2 * D], BF)
    din("bv", [1, D], BF)
    din("bproj", [1, D], BF)
    din("b1m", [1, DFF], BF)
    out_d = nc.dram_tensor("out", [3, NK, 128, S], F32, kind="ExternalOutput").ap()
    xa_scr = nc.dram_tensor("xa_scr", [3, NK, 128, S], F32).ap()
    xab_scr = nc.dram_tensor("xab_scr", [3, NK, 128, S], BF).ap()

    with tile.TileContext(nc) as tc:
        _emit(nc, tc, dram, out_d, xa_scr, xab_scr, with_bias)
    nc.compile()
    return nc


def _emit(nc, tc, dram, out_d, xa_scr, xab_scr, with_bias):
    from contextlib import ExitStack
    ctx = ExitStack()
    with ctx:
        pool_const = ctx.enter_context(tc.tile_pool(name="const", bufs=1))
        pool_rot = ctx.enter_context(tc.tile_pool(name="rot", bufs=2))
        pool_stat = ctx.enter_context(tc.tile_pool(name="stat", bufs=1))
        ps_big = ctx.enter_context(tc.tile_pool(name="psbig", bufs=5, space="PSUM"))
        ps_st = ctx.enter_context(tc.tile_pool(name="psst", bufs=3, space="PSUM"))

        # ---- constants ----
        ones1 = pool_const.tile([128, 128], BF, tag="ones1")
        nc.gpsimd.memset(ones1[:], 1.0)
        onesd = pool_const.tile([128, 128], BF, tag="onesd")
        nc.gpsimd.memset(onesd[:], 1.0 / D)
        onesrow = pool_const.tile([1, S], BF, tag="onesrow")
        nc.gpsimd.memset(onesrow[:], 1.0)
        epsb = pool_const.tile([128, 1], F32, tag="epsb")
        nc.gpsimd.memset(epsb[:], EPS)
        bqk_sb = pool_const.tile([1, 2 * D], BF, tag="bqk")
        nc.sync.dma_start(bqk_sb[:], dram["bqk"][:])
        bv_sb = pool_const.tile([1, D], BF, tag="bv")
        nc.sync.dma_start(bv_sb[:], dram["bv"][:])
        bproj_sb = pool_const.tile([1, D], BF, tag="bproj")
        nc.sync.dma_start(bproj_sb[:], dram["bproj"][:])
        b1m_sb = pool_const.tile([1, DFF], BF, tag="b1m")
        nc.sync.dma_start(b1m_sb[:], dram["b1m"][:])

        def ck(t, k):
            return t[:, k * S:(k + 1) * S]

        def f32tmp():
            return pool_rot.tile([128, S], F32, tag="f32tmp", bufs=3,
                                 name="f32tmp")

        # ---- LayerNorm: primal part (stats via PE colsum-broadcast) ----
        def ln_primal(in_bf, n_bf):
            mu_ps = ps_st.tile([128, S], F32, tag="st", name="mu_ps")
            s2_ps = ps_st.tile([128, S], F32, tag="st", name="s2_ps")
            for k in range(NK):
                sq = pool_rot.tile([128, S], BF, tag="sq", name="sq")
                nc.vector.tensor_mul(sq[:], ck(in_bf, k), ck(in_bf, k))
                nc.tensor.matmul(mu_ps[:], onesd[:], ck(in_bf, k),
                                 start=(k == 0), stop=(k == NK - 1))
                nc.tensor.matmul(s2_ps[:], onesd[:], sq[:],
                                 start=(k == 0), stop=(k == NK - 1))
            mu_f = pool_stat.tile([128, S], F32, tag="lnmu", name="mu_f")
            nc.scalar.copy(mu_f[:], mu_ps[:])
            mu2 = pool_rot.tile([128, S], BF, tag="sq", name="mu2")
            nc.vector.tensor_mul(mu2[:], mu_f[:], mu_f[:])
            var = f32tmp()
            nc.vector.tensor_sub(var[:], s2_ps[:], mu2[:])
            sd = f32tmp()
            nc.scalar.activation(sd[:], var[:], AF.Sqrt, bias=epsb[:])
            r_f = pool_stat.tile([128, S], F32, tag="lnr", name="r_f")
            nc.vector.reciprocal_approx_fast(r_f[:], sd[:])
            for k in range(NK):
                cen = pool_rot.tile([128, S], F32, tag="cen", name="cen")
                nc.vector.tensor_sub(cen[:], ck(in_bf, k), mu_f[:])
                nc.vector.tensor_mul(ck(n_bf, k), cen[:], r_f[:])
            return r_f

        # ---- LayerNorm: one tangent's JVP ----
        def ln_tangent(in_bf, tan_bf, n_bf, r_bf, dn_bf):  # r_bf is f32 now
            mt_ps = ps_st.tile([128, S], F32, tag="st", name="mt_ps")
            c_ps = ps_st.tile([128, S], F32, tag="st", name="c_ps")
            for k in range(NK):
                p = pool_rot.tile([128, S], BF, tag="p", name="p")
                nc.vector.tensor_mul(p[:], ck(n_bf, k), ck(tan_bf, k))
                nc.tensor.matmul(mt_ps[:], onesd[:], ck(tan_bf, k),
                                 start=(k == 0), stop=(k == NK - 1))
                nc.tensor.matmul(c_ps[:], onesd[:], p[:],
                                 start=(k == 0), stop=(k == NK - 1))
            ctr = pool_rot.tile([128, S], BF, tag="ctr", name="ctr")
            nc.vector.tensor_mul(ctr[:], c_ps[:], r_bf[:])
            mt_f = pool_rot.tile([128, S], F32, tag="mtf", name="mt_f")
            nc.scalar.copy(mt_f[:], mt_ps[:])
            for k in range(NK):
                b_ = pool_rot.tile([128, S], F32, tag="cen", name="b_")
                nc.vector.tensor_sub(b_[:], ck(tan_bf, k), mt_f[:])
                e_ = pool_rot.tile([128, S], F32, tag="e", name="e_")
                nc.vector.tensor_mul(e_[:], b_[:], r_bf[:])
                f_ = pool_rot.tile([128, S], BF, tag="f", name="f_")
                nc.vector.tensor_mul(f_[:], ck(n_bf, k), ctr[:])
                nc.vector.tensor_sub(ck(dn_bf, k), e_[:], f_[:])

        # LN input/output pools outlive the o_sb pool (LIFO nesting); the
        # LN2 tensors reuse the LN1 tags (slot reuse after LN1 consumers end).
        st_n2 = ExitStack()
        pool_lnout = st_n2.enter_context(tc.tile_pool(name="lnout", bufs=1))
        st_ab = ExitStack()
        pool_lnio = st_ab.enter_context(tc.tile_pool(name="lnio", bufs=1))

        st_o = ExitStack()
        pool_o = st_o.enter_context(tc.tile_pool(name="osb", bufs=1))
        o_sb = pool_o.tile([128, NK * S], BF, tag="o", name="o_sb")
        do_sb = [pool_o.tile([128, NK * S], BF, tag=f"do{m}", name=f"do_sb{m}")
                 for m in range(2)]

        st_qkv = ExitStack()
        pool_qkv = st_qkv.enter_context(tc.tile_pool(name="qkv", bufs=1))
        q_sb = pool_qkv.tile([128, NK * S], BF, tag="q", name="q_sb")
        k_sb = pool_qkv.tile([128, NK * S], BF, tag="k", name="k_sb")
        dq_sb = [pool_qkv.tile([128, NK * S], BF, tag=f"dq{m}", name=f"dq_sb{m}")
                 for m in range(2)]
        dk_sb = [pool_qkv.tile([128, NK * S], BF, tag=f"dk{m}", name=f"dk_sb{m}")
                 for m in range(2)]
        v_sb = pool_qkv.tile([128, NSC * D], BF, tag="v", name="v_sb")
        dv_sb = [pool_qkv.tile([128, NSC * D], BF, tag=f"dv{m}", name=f"dv_sb{m}")
                 for m in range(2)]

        # =========== Phases A+B: LN1 and QKV, primal first ===========
        with tc.tile_pool(name="wq", bufs=1) as pool_wq:
            n1 = pool_lnout.tile([128, NK * S], BF, tag="n1", name="n1")
            dn1 = [pool_lnout.tile([128, NK * S], BF, tag=f"dn1{m}",
                                   name=f"dn1{m}") for m in range(2)]
            xbf = pool_lnio.tile([128, NK * S], BF, tag="xbf", name="xbf")
            tbf = [pool_lnio.tile([128, NK * S], BF, tag=f"tbf{m}",
                                  name=f"tbf{m}") for m in range(2)]
            for k in range(NK):
                nc.scalar.dma_start(ck(xbf, k), dram["x_bf"][k])
                for m in range(2):
                    nc.scalar.dma_start(ck(tbf[m], k), dram["t_bf"][m, k])
            wqkv_t = []
            for k in range(NK):
                wt = pool_wq.tile([128, 3 * D], BF, tag=f"wqkv{k}",
                                  name=f"wqkv{k}")
                nc.sync.dma_start(wt[:], dram["wqkv"][k])
                wqkv_t.append(wt)

            r1_bf = ln_primal(xbf, n1)

            def qk_pass2(src, qd, kd, with_bias):
                for mt in range(12):
                    ps = ps_big.tile([128, S], F32, tag="big", name="qkv_ps")
                    for k in range(NK):
                        nc.tensor.matmul(ps[:],
                                         wqkv_t[k][:, mt * 128:(mt + 1) * 128],
                                         ck(src, k), start=(k == 0),
                                         stop=(k == NK - 1 and not with_bias))
                    if with_bias:
                        nc.tensor.matmul(ps[:],
                                         bqk_sb[0:1, mt * 128:(mt + 1) * 128],
                                         onesrow[:], start=False, stop=True)
                    if mt < 6:
                        nc.scalar.mul(qd[:, mt * S:(mt + 1) * S], ps[:], SCALE)
                    else:
                        nc.scalar.copy(kd[:, (mt - 6) * S:(mt - 5) * S], ps[:])

            def v_pass(src, vd, with_bias):
                for sc in range(NSC):
                    for g2 in range(2):
                        wv_col = 2 * D + g2 * 384
                        ps = ps_big.tile([128, 384], F32, tag="big", name="v_ps")
                        for k in range(NK):
                            lhs = src[:, k * S + sc * 128: k * S + (sc + 1) * 128]
                            nc.tensor.matmul(ps[:], lhs,
                                             wqkv_t[k][:, wv_col:wv_col + 384],
                                             start=(k == 0),
                                             stop=(k == NK - 1 and not with_bias))
                        if with_bias:
                            nc.tensor.matmul(ps[:], ones1[0:1, :],
                                             bv_sb[0:1, g2 * 384:(g2 + 1) * 384],
                                             start=False, stop=True)
                        col = sc * D + g2 * 384
                        nc.scalar.copy(vd[:, col:col + 384], ps[:])

            qk_pass2(n1, q_sb, k_sb, with_bias)
            v_pass(n1, v_sb, with_bias)
            for m in range(2):
                ln_tangent(xbf, tbf[m], n1, r1_bf, dn1[m])
            for m in range(2):
                qk_pass2(dn1[m], dq_sb[m], dk_sb[m], False)
                v_pass(dn1[m], dv_sb[m], False)

        # =========== Phase C: attention, head pairs ===========
        # Heads are processed in pairs (2i, 2i+1) sharing one feature chunk:
        # the K=64 score matmuls of the two heads run concurrently in the PE
        # via row tile_position (0,0)/(64,0); the M=64 attention-value matmuls
        # share one PSUM bank via column tile_position (0,0)/(0,64).
        # Two-stage software pipeline as before, one pair per iteration.
        with tc.tile_pool(name="head2", bufs=2) as pool_h2, \
             tc.tile_pool(name="head1", bufs=1) as pool_h1:
            def stage1(pi):
                kc = pi
                qp = q_sb[:, kc * S:(kc + 1) * S]
                kp = k_sb[:, kc * S:(kc + 1) * S]
                sT = {}
                expT = [pool_h2.tile([128, NSC * S], BF, tag=f"exp{j}",
                                     name=f"expT{j}") for j in range(2)]
                for c in range(NSC):
                    for j, po in ((0, 0), (1, 64)):
                        st = ps_big.tile([128, S], F32, tag="big", name="sT")
                        nc.tensor.matmul(st[:],
                                         kp[po:po + 64, c * 128:(c + 1) * 128],
                                         qp[po:po + 64, :], start=True,
                                         stop=True, tile_position=(po, 0))
                        sT[(j, c)] = st
                    for j in range(2):
                        nc.scalar.activation(ck(expT[j], c), sT[(j, c)][:],
                                             AF.Exp)
                ds_sb = []
                for m in range(2):
                    dqp = dq_sb[m][:, kc * S:(kc + 1) * S]
                    dkp = dk_sb[m][:, kc * S:(kc + 1) * S]
                    dsb = [pool_h2.tile([128, NSC * S], BF, tag=f"ds{m}{j}",
                                        name=f"ds_sb{m}{j}", bufs=1)
                           for j in range(2)]
                    for c in range(NSC):
                        dsp = {}
                        for j, po in ((0, 0), (1, 64)):
                            ps = ps_big.tile([128, S], F32, tag="big",
                                             name="dsT")
                            nc.tensor.matmul(
                                ps[:], dkp[po:po + 64, c * 128:(c + 1) * 128],
                                qp[po:po + 64, :], start=True, stop=False,
                                tile_position=(po, 0))
                            nc.tensor.matmul(
                                ps[:], kp[po:po + 64, c * 128:(c + 1) * 128],
                                dqp[po:po + 64, :], start=False, stop=True,
                                tile_position=(po, 0))
                            dsp[j] = ps
                        for j in range(2):
                            if m == 0:
                                nc.scalar.copy(ck(dsb[j], c), dsp[j][:])
                            else:
                                nc.vector.tensor_copy(ck(dsb[j], c), dsp[j][:])
                    ds_sb.append(dsb)
                return expT, ds_sb

            def stage2(pi, expT, ds_sb):
                kc = pi
                sums = [ps_st.tile([128, S], F32, tag="st", name="sums")
                        for _ in range(2)]
                for j in range(2):
                    for c in range(NSC):
                        nc.tensor.matmul(sums[j][:], ones1[:], ck(expT[j], c),
                                         start=(c == 0), stop=(c == NSC - 1))
                rinv = []
                for j in range(2):
                    rf = f32tmp()
                    nc.vector.reciprocal_approx_fast(rf[:], sums[j][:])
                    rv = pool_h1.tile([128, S], BF, tag=f"rinv{j}",
                                      name=f"rinv{j}")
                    nc.scalar.copy(rv[:], rf[:])
                    rinv.append(rv)
                at = [pool_h1.tile([128, NSC * S], BF, tag=f"at{j}",
                                   name=f"at{j}") for j in range(2)]
                for j in range(2):
                    for c in range(NSC):
                        nc.vector.tensor_mul(ck(at[j], c), ck(expT[j], c),
                                             rinv[j][:])
                o_ps = ps_st.tile([128, S], F32, tag="st", name="o_ps")
                for c in range(NSC):
                    for j, po in ((0, 0), (1, 64)):
                        h = 2 * pi + j
                        vh = v_sb[:, c * D + h * HD: c * D + (h + 1) * HD]
                        nc.tensor.matmul(o_ps[po:po + 64, :], vh, ck(at[j], c),
                                         start=(c == 0), stop=(c == NSC - 1),
                                         tile_position=(0, po))
                osl = o_sb[:, kc * S:(kc + 1) * S]
                nc.scalar.copy(osl, o_ps[:])
                for m in range(2):
                    pt = [pool_h1.tile([128, NSC * S], BF, tag=f"pt{j}",
                                       name=f"pt{j}") for j in range(2)]
                    for j in range(2):
                        for c in range(NSC):
                            nc.vector.tensor_mul(ck(pt[j], c), ck(at[j], c),
                                                 ck(ds_sb[m][j], c))
                    c_ps = [ps_st.tile([128, S], F32, tag="st", name="c_ps2")
                            for _ in range(2)]
                    for j in range(2):
                        for c in range(NSC):
                            nc.tensor.matmul(c_ps[j][:], ones1[:],
                                             ck(pt[j], c), start=(c == 0),
                                             stop=(c == NSC - 1))
                    do_ps = ps_st.tile([128, S], F32, tag="st", name="do_ps")
                    for c in range(NSC):
                        for j, po in ((0, 0), (1, 64)):
                            h = 2 * pi + j
                            vh = v_sb[:, c * D + h * HD: c * D + (h + 1) * HD]
                            dvh = dv_sb[m][:, c * D + h * HD:
                                           c * D + (h + 1) * HD]
                            nc.tensor.matmul(do_ps[po:po + 64, :], vh,
                                             ck(pt[j], c), start=(c == 0),
                                             stop=False, tile_position=(0, po))
                            nc.tensor.matmul(do_ps[po:po + 64, :], dvh,
                                             ck(at[j], c), start=False,
                                             stop=(c == NSC - 1),
                                             tile_position=(0, po))
                    corr = pool_rot.tile([128, S], F32, tag="corr", name="corr", bufs=1)
                    for j, po in ((0, 0), (1, 64)):
                        nc.vector.tensor_mul(corr[po:po + 64, :],
                                             o_sb[po:po + 64,
                                                  kc * S:(kc + 1) * S],
                                             c_ps[j][po:po + 64, :])
                    nc.vector.tensor_sub(do_sb[m][:, kc * S:(kc + 1) * S],
                                         do_ps[:], corr[:])

            pend = None
            for pi in range(H // 2):
                args = stage1(pi)
                if pend is not None:
                    stage2(*pend)
                pend = (pi, *args)
            stage2(*pend)
        st_qkv.close()

        # =========== Phase D: proj + residual (spill xa f32 to DRAM) ========
        with tc.tile_pool(name="wp", bufs=1) as pool_wp, \
             tc.tile_pool(name="resid", bufs=1) as pool_res:
            # warm the sqrt table set while the PE is still busy, so LN2's
            # critical path doesn't pay the ACT table load
            warm = pool_rot.tile([128, 1], F32, tag="warm", name="warm")
            nc.scalar.activation(warm[:], epsb[:], AF.Sqrt, bias=epsb[:])
            wproj_t = []
            for k in range(NK):
                wt = pool_wp.tile([128, D], BF, tag=f"wp{k}", name=f"wp{k}")
                nc.sync.dma_start(wt[:], dram["wproj"][k])
                wproj_t.append(wt)
            # prefetch all residual inputs
            res_in = pool_res.tile([128, 3 * NK * S], F32, tag="resin",
                                   name="res_in")
            for si in range(3):
                for k in range(NK):
                    dst = res_in[:, (si * NK + k) * S:(si * NK + k + 1) * S]
                    if si == 0:
                        nc.scalar.dma_start(dst, dram["x_f32"][k])
                    else:
                        nc.sync.dma_start(dst, dram["t_f32"][si - 1, k])
            xabf = pool_lnio.tile([128, NK * S], BF, tag="xbf", name="xabf")
            dxabf = [pool_lnio.tile([128, NK * S], BF, tag=f"tbf{m}",
                                    name=f"dxabf{m}") for m in range(2)]
            psrcs = [o_sb, do_sb[0], do_sb[1]]
            pdsts = [xabf, dxabf[0], dxabf[1]]
            for mt in range(NK):
                for si in range(3):
                    ps = ps_big.tile([128, S], F32, tag="big", name="pj_ps")
                    for k in range(NK):
                        nc.tensor.matmul(ps[:],
                                         wproj_t[k][:, mt * 128:(mt + 1) * 128],
                                         ck(psrcs[si], k), start=(k == 0),
                                         stop=(k == NK - 1 and
                                               (si != 0 or not with_bias)))
                    if si == 0 and with_bias:
                        nc.tensor.matmul(
                            ps[:], bproj_sb[0:1, mt * 128:(mt + 1) * 128],
                            onesrow[:], start=False, stop=True)
                    xaf = f32tmp()
                    nc.vector.tensor_add(
                        xaf[:], res_in[:, (si * NK + mt) * S:
                                       (si * NK + mt + 1) * S], ps[:])
                    nc.sync.dma_start(xa_scr[si, mt], xaf[:])
                    nc.scalar.copy(ck(pdsts[si], mt), xaf[:])
        st_o.close()

        # =========== Phase E: LN2 (primal first) ===========
        n2 = pool_lnout.tile([128, NK * S], BF, tag="n1", name="n2")
        dn2 = [pool_lnout.tile([128, NK * S], BF, tag=f"dn1{m}", name=f"dn2{m}")
               for m in range(2)]
        r2_bf = ln_primal(xabf, n2)
        ln_tangent(xabf, dxabf[0], n2, r2_bf, dn2[0])

        # =========== Phase F1: W1 + gelu, primal first ===========
        st_g = ExitStack()
        pool_g = st_g.enter_context(tc.tile_pool(name="gq", bufs=1))
        g_sb = pool_g.tile([128, NM1 * S], BF, tag="g", name="g_sb")
        q2_sb = [pool_g.tile([128, NM1 * S], BF, tag=f"q2{m}", name=f"q2_sb{m}")
                 for m in range(2)]
        with tc.tile_pool(name="dgp", bufs=1) as pool_dg:
            for half in range(2):
                dg_sb = pool_dg.tile([128, (NM1 // 2) * S], BF, tag="dg",
                                     name="dg_sb")
                with tc.tile_pool(name=f"w1h{half}", bufs=1) as pool_w1:
                    w1_t = []
                    for k in range(NK):
                        wt = pool_w1.tile([128, DFF // 2], BF, tag=f"w1{k}",
                                          name=f"w1_{half}_{k}")
                        nc.sync.dma_start(
                            wt[:], dram["w1"][k][:, half * (DFF // 2):
                                                 (half + 1) * (DFF // 2)])
                        w1_t.append(wt)
                    for mi in range(NM1 // 2):
                        mt = half * (NM1 // 2) + mi
                        ps = ps_big.tile([128, S], F32, tag="big", name="u_ps")
                        for k in range(NK):
                            nc.tensor.matmul(ps[:],
                                             w1_t[k][:, mi * 128:(mi + 1) * 128],
                                             ck(n2, k), start=(k == 0),
                                             stop=(k == NK - 1 and
                                                   not with_bias))
                        if with_bias:
                            nc.tensor.matmul(
                                ps[:], b1m_sb[0:1, mt * 128:(mt + 1) * 128],
                                onesrow[:], start=False, stop=True)
                        nc.scalar.activation(ck(g_sb, mt), ps[:], AF.Gelu)
                        nc.scalar.activation(ck(dg_sb, mi), ps[:],
                                             AF.Derivative_Gelu)
                    if half == 0:
                        ln_tangent(xabf, dxabf[1], n2, r2_bf, dn2[1])
                    for m in range(2):
                        for mi in range(NM1 // 2):
                            mt = half * (NM1 // 2) + mi
                            ps = ps_big.tile([128, S], F32, tag="big",
                                             name="ut_ps")
                            for k in range(NK):
                                nc.tensor.matmul(
                                    ps[:], w1_t[k][:, mi * 128:(mi + 1) * 128],
                                    ck(dn2[m], k), start=(k == 0),
                                    stop=(k == NK - 1))
                            nc.vector.tensor_mul(ck(q2_sb[m], mt),
                                                 ck(dg_sb, mi), ps[:])

        # =========== Phase F2: W2 + final residual ===========
        fsrcs = [g_sb, q2_sb[0], q2_sb[1]]
        with tc.tile_pool(name="w2p", bufs=1) as pool_w2, \
             tc.tile_pool(name="res2", bufs=1) as pool_res2:
            w2_t = []
            for k in range(NM1):
                wt = pool_w2.tile([128, D], BF, tag=f"w2{k}", name=f"w2_{k}")
                nc.scalar.dma_start(wt[:], dram["w2"][k])
                w2_t.append(wt)
            for si in range(3):
                for mt in range(NK):
                    res2 = pool_res2.tile([128, S], F32, tag="res2", bufs=6,
                                          name="res2")
                    nc.sync.dma_start(res2[:], xa_scr[si, mt])
                    ps = ps_big.tile([128, S], F32, tag="big", name="o2_ps")
                    for k in range(NM1):
                        nc.tensor.matmul(ps[:],
                                         w2_t[k][:, mt * 128:(mt + 1) * 128],
                                         ck(fsrcs[si], k), start=(k == 0),
                                         stop=(k == NM1 - 1))
                    ot = f32tmp()
                    nc.vector.tensor_add(ot[:], res2[:], ps[:])
                    nc.sync.dma_start(out_d[si, mt], ot[:])
        st_g.close()
        st_ab.close()
        st_n2.close()


def _prep_host(inputs):
    f32 = np.float32
    x = np.asarray(inputs["x"], f32)
    xt = np.asarray(inputs["x_tangent"], f32)
    g1 = np.asarray(inputs["g1"], f32); b1 = np.asarray(inputs["b1"], f32)
    g2 = np.asarray(inputs["g2"], f32); b2 = np.asarray(inputs["b2"], f32)
    Wqkv = np.asarray(inputs["Wqkv"], f32); Wproj = np.asarray(inputs["Wproj"], f32)
    W1 = np.asarray(inputs["W1"], f32); W2 = np.asarray(inputs["W2"], f32)
    bproj = np.asarray(inputs["bproj"], f32)
    bf1 = np.asarray(inputs["bf1"], f32); bf2 = np.asarray(inputs["bf2"], f32)

    bf16 = ml_dtypes.bfloat16
    Wqkv_f = g1[:, None] * Wqkv
    bqkv = b1 @ Wqkv
    W1_f = g2[:, None] * W1
    b1m = b2 @ W1 + bf1

    def tile_k(w, nk):
        return np.ascontiguousarray(w.reshape(nk, 128, -1))

    shared = {
        "wqkv": tile_k(Wqkv_f, NK).astype(bf16),
        "wproj": tile_k(Wproj, NK).astype(bf16),
        "w1": tile_k(W1_f, NK).astype(bf16),
        "w2": tile_k(W2, NM1).astype(bf16),
        "bqk": np.ascontiguousarray(bqkv[None, :2 * D]).astype(bf16),
        "bv": np.ascontiguousarray(bqkv[None, 2 * D:]).astype(bf16),
        "bproj": np.ascontiguousarray(bproj[None, :]).astype(bf16),
        "b1m": np.ascontiguousarray(b1m[None, :]).astype(bf16),
    }
    in_maps = []
    for core in range(N_CORES):
        b, mp = core // 2, core % 2
        xT = np.ascontiguousarray(x[b].T).reshape(NK, 128, S)
        tT = np.ascontiguousarray(
            xt[b, 2 * mp:2 * mp + 2].transpose(0, 2, 1)).reshape(2, NK, 128, S)
        im = dict(shared)
        im["x_f32"] = xT
        im["x_bf"] = xT.astype(bf16)
        im["t_f32"] = tT
        im["t_bf"] = tT.astype(bf16)
        in_maps.append(im)
    return in_maps, bf2


def kernel(**inputs):
    with_bias = not all(
        np.allclose(np.asarray(inputs[k]), 0.0)
        for k in ("b1", "bproj", "b2", "bf1"))
    key = ("nc", with_bias)
    if key not in _CACHE:
        _CACHE[key] = _build_program(with_bias)
    nc = _CACHE[key]
    in_maps, bf2 = _prep_host(inputs)
    res = run_bass_kernel_spmd(nc, in_maps, core_ids=list(range(N_CORES)),
                               **_RUN_KWARGS)
    _LAST_RES[0] = res
    out = np.zeros((B, S, D), np.float32)
    out_tan = np.zeros((B, M, S, D), np.float32)
    for core in range(N_CORES):
        b, mp = core // 2, core % 2
        o = res.results[core]["out"].reshape(3, D, S)
        if mp == 0:
            out[b] = o[0].T + bf2[None, :]
        out_tan[b, 2 * mp] = o[1].T
        out_tan[b, 2 * mp + 1] = o[2].T
    return out, out_tan


# revision 23
# speedup vs baseline: 1.2029x; 1.0029x over previous
# Trainium2 Bass kernel for a pre-norm transformer block with forward-mode JVP
# (jax.linearize) over M=4 tangent directions.
#
# Sharding: 8 cores; core c handles batch b=c//2 and tangents {2*(c%2), 2*(c%2)+1}.
# Each core computes the primal pass for its batch (even/odd core pairs do this
# redundantly; the even core's primal is used) plus 2 tangent JVP passes.
# No cross-core communication.
#
# On-chip layout is feature-major ([D, S] with features on partitions), so every
# linear layer is a plain accumulated matmul with no transposes. LayerNorm /
# softmax statistics (which reduce over partitions in this layout) are computed
# on the PE via ones-matmuls whose M=128 output broadcasts the column sums to
# all partitions. Softmax is computed without max-subtraction (scores are O(3)
# for this problem's data distribution). LN affine (g, b) is folded into the
# weights on the host; biases enter via K=1 ones-row matmuls; the final mlp
# bias bf2 is added on the host.
#
# Matmul inputs are bf16 (fp32 PSUM accumulation); the residual stream and LN /
# softmax statistics stay fp32 (residual adds read the fp32 inputs re-DMAed
# from DRAM; xa is spilled to DRAM scratch between the attention and MLP
# residual adds to stay under the SBUF budget).

import numpy as np
import ml_dtypes

import concourse.bass as bass
import concourse.tile as tile
from concourse import bacc, mybir
from concourse.bass_utils import run_bass_kernel_spmd

AF = mybir.ActivationFunctionType
BF = mybir.dt.bfloat16
F32 = mybir.dt.float32

B, S, D, H, M = 4, 512, 768, 12, 4
DFF = 4 * D
HD = D // H
EPS = 1e-6
SCALE = HD ** -0.5
NK = D // 128          # 6 feature chunks
NSC = S // 128         # 4 sequence chunks
NM1 = DFF // 128       # 24
N_CORES = 8

_CACHE = {}
_RUN_KWARGS = {}   # test harness can set {"trace": True}
_LAST_RES = [None]


def _build_program(with_bias):
    nc = bacc.Bacc("TRN2", target_bir_lowering=False, debug=False,
                   num_devices=N_CORES)

    dram = {}
    def din(name, shape, dt):
        dram[name] = nc.dram_tensor(name, shape, dt, kind="ExternalInput").ap()
    din("x_f32", [NK, 128, S], F32)
    din("x_bf", [NK, 128, S], BF)
    din("t_f32", [2, NK, 128, S], F32)
    din("t_bf", [2, NK, 128, S], BF)
    din("wqkv", [NK, 128, 3 * D], BF)
    din("wproj", [NK, 128, D], BF)
    din("w1", [NK, 128, DFF], BF)
    din("w2", [NM1, 128, D], BF)
    din("bqk", [1, 2 * D], BF)
    din("bv", [1, D], BF)
    din("bproj", [1, D], BF)
    din("b1m", [1, DFF], BF)
    out_d = nc.dram_tensor("out", [3, NK, 128, S], F32, kind="ExternalOutput").ap()
    xa_scr = nc.dram_tensor("xa_scr", [3, NK, 128, S], F32).ap()
    xab_scr = nc.dram_tensor("xab_scr", [3, NK, 128, S], BF).ap()

    with tile.TileContext(nc) as tc:
        _emit(nc, tc, dram, out_d, xa_scr, xab_scr, with_bias)
    nc.compile()
    return nc


def _emit(nc, tc, dram, out_d, xa_scr, xab_scr, with_bias):
    from contextlib import ExitStack
    ctx = ExitStack()
    with ctx:
        pool_const = ctx.enter_context(tc.tile_pool(name="const", bufs=1))
        pool_rot = ctx.enter_context(tc.tile_pool(name="rot", bufs=2))
        pool_stat = ctx.enter_context(tc.tile_pool(name="stat", bufs=1))
        ps_big = ctx.enter_context(tc.tile_pool(name="psbig", bufs=5, space="PSUM"))
        ps_st = ctx.enter_context(tc.tile_pool(name="psst", bufs=3, space="PSUM"))

        # ---- constants ----
        ones1 = pool_const.tile([128, 128], BF, tag="ones1")
        nc.gpsimd.memset(ones1[:], 1.0)
        onesd = pool_const.tile([128, 128], BF, tag="onesd")
        nc.gpsimd.memset(onesd[:], 1.0 / D)
        onesrow = pool_const.tile([1, S], BF, tag="onesrow")
        nc.gpsimd.memset(onesrow[:], 1.0)
        epsb = pool_const.tile([128, 1], F32, tag="epsb")
        nc.gpsimd.memset(epsb[:], EPS)
        bqk_sb = pool_const.tile([1, 2 * D], BF, tag="bqk")
        nc.sync.dma_start(bqk_sb[:], dram["bqk"][:])
        bv_sb = pool_const.tile([1, D], BF, tag="bv")
        nc.sync.dma_start(bv_sb[:], dram["bv"][:])
        bproj_sb = pool_const.tile([1, D], BF, tag="bproj")
        nc.sync.dma_start(bproj_sb[:], dram["bproj"][:])
        b1m_sb = pool_const.tile([1, DFF], BF, tag="b1m")
        nc.sync.dma_start(b1m_sb[:], dram["b1m"][:])

        warm0 = pool_const.tile([128, 1], F32, tag="warm0")
        nc.scalar.activation(warm0[:], epsb[:], AF.Sqrt, bias=epsb[:])

        def ck(t, k):
            return t[:, k * S:(k + 1) * S]

        def f32tmp():
            return pool_rot.tile([128, S], F32, tag="f32tmp", bufs=3,
                                 name="f32tmp")

        # ---- LayerNorm: primal part (stats via PE colsum-broadcast) ----
        def ln_primal(in_bf, n_bf):
            mu_ps = ps_st.tile([128, S], F32, tag="st", name="mu_ps")
            s2_ps = ps_st.tile([128, S], F32, tag="st", name="s2_ps")
            for k in range(NK):
                sq = pool_rot.tile([128, S], BF, tag="sq", name="sq")
                nc.vector.tensor_mul(sq[:], ck(in_bf, k), ck(in_bf, k))
                nc.tensor.matmul(mu_ps[:], onesd[:], ck(in_bf, k),
                                 start=(k == 0), stop=(k == NK - 1))
                nc.tensor.matmul(s2_ps[:], onesd[:], sq[:],
                                 start=(k == 0), stop=(k == NK - 1))
            mu_f = pool_stat.tile([128, S], F32, tag="lnmu", name="mu_f")
            nc.scalar.copy(mu_f[:], mu_ps[:])
            mu2 = pool_rot.tile([128, S], BF, tag="sq", name="mu2")
            nc.vector.tensor_mul(mu2[:], mu_f[:], mu_f[:])
            var = f32tmp()
            nc.vector.tensor_sub(var[:], s2_ps[:], mu2[:])
            sd = f32tmp()
            nc.scalar.activation(sd[:], var[:], AF.Sqrt, bias=epsb[:])
            r_f = pool_stat.tile([128, S], F32, tag="lnr", name="r_f")
            nc.vector.reciprocal_approx_fast(r_f[:], sd[:])
            for k in range(NK):
                cen = pool_rot.tile([128, S], F32, tag="cen", name="cen")
                nc.vector.tensor_sub(cen[:], ck(in_bf, k), mu_f[:])
                nc.vector.tensor_mul(ck(n_bf, k), cen[:], r_f[:])
            return r_f

        # ---- LayerNorm: one tangent's JVP ----
        def ln_tangent(in_bf, tan_bf, n_bf, r_bf, dn_bf):  # r_bf is f32 now
            mt_ps = ps_st.tile([128, S], F32, tag="st", name="mt_ps")
            c_ps = ps_st.tile([128, S], F32, tag="st", name="c_ps")
            for k in range(NK):
                p = pool_rot.tile([128, S], BF, tag="p", name="p")
                nc.vector.tensor_mul(p[:], ck(n_bf, k), ck(tan_bf, k))
                nc.tensor.matmul(mt_ps[:], onesd[:], ck(tan_bf, k),
                                 start=(k == 0), stop=(k == NK - 1))
                nc.tensor.matmul(c_ps[:], onesd[:], p[:],
                                 start=(k == 0), stop=(k == NK - 1))
            ctr = pool_rot.tile([128, S], BF, tag="ctr", name="ctr")
            nc.vector.tensor_mul(ctr[:], c_ps[:], r_bf[:])
            mt_f = pool_rot.tile([128, S], F32, tag="f32tmp", bufs=3, name="mt_f")
            nc.scalar.copy(mt_f[:], mt_ps[:])
            for k in range(NK):
                b_ = pool_rot.tile([128, S], F32, tag="cen", name="b_")
                nc.vector.tensor_sub(b_[:], ck(tan_bf, k), mt_f[:])
                e_ = pool_rot.tile([128, S], F32, tag="e", name="e_")
                nc.vector.tensor_mul(e_[:], b_[:], r_bf[:])
                f_ = pool_rot.tile([128, S], BF, tag="f", name="f_")
                nc.vector.tensor_mul(f_[:], ck(n_bf, k), ctr[:])
                nc.vector.tensor_sub(ck(dn_bf, k), e_[:], f_[:])

        # LN input/output pools outlive the o_sb pool (LIFO nesting); the
        # LN2 tensors reuse the LN1 tags (slot reuse after LN1 consumers end).
        st_n2 = ExitStack()
        pool_lnout = st_n2.enter_context(tc.tile_pool(name="lnout", bufs=1))
        st_ab = ExitStack()
        pool_lnio = st_ab.enter_context(tc.tile_pool(name="lnio", bufs=1))

        st_o = ExitStack()
        pool_o = st_o.enter_context(tc.tile_pool(name="osb", bufs=1))
        o_sb = pool_o.tile([128, NK * S], BF, tag="o", name="o_sb")
        do_sb = [pool_o.tile([128, NK * S], BF, tag=f"do{m}", name=f"do_sb{m}")
                 for m in range(2)]

        st_wp = ExitStack()
        pool_wp = st_wp.enter_context(tc.tile_pool(name="wp", bufs=1))
        wproj_t = []
        for k in range(NK // 2):
            wt = pool_wp.tile([128, D], BF, tag=f"wp{k}", name=f"wp{k}")
            nc.sync.dma_start(wt[:], dram["wproj"][k])
            wproj_t.append(wt)

        st_qkv = ExitStack()
        pool_qkv = st_qkv.enter_context(tc.tile_pool(name="qkv", bufs=1))
        q_sb = pool_qkv.tile([128, NK * S], BF, tag="q", name="q_sb")
        k_sb = pool_qkv.tile([128, NK * S], BF, tag="k", name="k_sb")
        dq_sb = [pool_qkv.tile([128, NK * S], BF, tag=f"dq{m}", name=f"dq_sb{m}")
                 for m in range(2)]
        dk_sb = [pool_qkv.tile([128, NK * S], BF, tag=f"dk{m}", name=f"dk_sb{m}")
                 for m in range(2)]
        v_sb = pool_qkv.tile([128, NSC * D], BF, tag="v", name="v_sb")
        dv_sb = [pool_qkv.tile([128, NSC * D], BF, tag=f"dv{m}", name=f"dv_sb{m}")
                 for m in range(2)]

        # =========== Phases A+B: LN1 and QKV, primal first ===========
        with tc.tile_pool(name="wq", bufs=1) as pool_wq:
            n1 = pool_lnout.tile([128, NK * S], BF, tag="n1", name="n1")
            dn1 = [pool_lnout.tile([128, NK * S], BF, tag=f"dn1{m}",
                                   name=f"dn1{m}") for m in range(2)]
            xbf = pool_lnio.tile([128, NK * S], BF, tag="xbf", name="xbf")
            tbf = [pool_lnio.tile([128, NK * S], BF, tag=f"tbf{m}",
                                  name=f"tbf{m}") for m in range(2)]
            for k in range(NK):
                nc.scalar.dma_start(ck(xbf, k), dram["x_bf"][k])
                for m in range(2):
                    nc.scalar.dma_start(ck(tbf[m], k), dram["t_bf"][m, k])
            wqkv_t = []
            for k in range(NK):
                wt = pool_wq.tile([128, 3 * D], BF, tag=f"wqkv{k}",
                                  name=f"wqkv{k}")
                nc.sync.dma_start(wt[:], dram["wqkv"][k])
                wqkv_t.append(wt)

            r1_bf = ln_primal(xbf, n1)

            def qk_pass2(src, qd, kd, with_bias):
                for mt in range(12):
                    ps = ps_big.tile([128, S], F32, tag="big", name="qkv_ps")
                    for k in range(NK):
                        nc.tensor.matmul(ps[:],
                                         wqkv_t[k][:, mt * 128:(mt + 1) * 128],
                                         ck(src, k), start=(k == 0),
                                         stop=(k == NK - 1 and not with_bias))
                    if with_bias:
                        nc.tensor.matmul(ps[:],
                                         bqk_sb[0:1, mt * 128:(mt + 1) * 128],
                                         onesrow[:], start=False, stop=True)
                    if mt < 6:
                        nc.scalar.mul(qd[:, mt * S:(mt + 1) * S], ps[:], SCALE)
                    else:
                        nc.scalar.copy(kd[:, (mt - 6) * S:(mt - 5) * S], ps[:])

            def v_pass(src, vd, with_bias):
                for sc in range(NSC):
                    for g2 in range(2):
                        wv_col = 2 * D + g2 * 384
                        ps = ps_big.tile([128, 384], F32, tag="big", name="v_ps")
                        for k in range(NK):
                            lhs = src[:, k * S + sc * 128: k * S + (sc + 1) * 128]
                            nc.tensor.matmul(ps[:], lhs,
                                             wqkv_t[k][:, wv_col:wv_col + 384],
                                             start=(k == 0),
                                             stop=(k == NK - 1 and not with_bias))
                        if with_bias:
                            nc.tensor.matmul(ps[:], ones1[0:1, :],
                                             bv_sb[0:1, g2 * 384:(g2 + 1) * 384],
                                             start=False, stop=True)
                        col = sc * D + g2 * 384
                        nc.scalar.copy(vd[:, col:col + 384], ps[:])

            qk_pass2(n1, q_sb, k_sb, with_bias)
            v_pass(n1, v_sb, with_bias)
            for m in range(2):
                ln_tangent(xbf, tbf[m], n1, r1_bf, dn1[m])
            for m in range(2):
                qk_pass2(dn1[m], dq_sb[m], dk_sb[m], False)
                v_pass(dn1[m], dv_sb[m], False)

        # =========== Phase C: attention, head pairs ===========
        # Heads are processed in pairs (2i, 2i+1) sharing one feature chunk:
        # the K=64 score matmuls of the two heads run concurrently in the PE
        # via row tile_position (0,0)/(64,0); the M=64 attention-value matmuls
        # share one PSUM bank via column tile_position (0,0)/(0,64).
        # Two-stage software pipeline as before, one pair per iteration.
        with tc.tile_pool(name="head2", bufs=2) as pool_h2, \
             tc.tile_pool(name="head1", bufs=1) as pool_h1:
            def stage1(pi):
                kc = pi
                qp = q_sb[:, kc * S:(kc + 1) * S]
                kp = k_sb[:, kc * S:(kc + 1) * S]
                sT = {}
                expT = [pool_h2.tile([128, NSC * S], BF, tag=f"exp{j}",
                                     name=f"expT{j}") for j in range(2)]
                for c in range(NSC):
                    for j, po in ((0, 0), (1, 64)):
                        st = ps_big.tile([128, S], F32, tag="big", name="sT")
                        nc.tensor.matmul(st[:],
                                         kp[po:po + 64, c * 128:(c + 1) * 128],
                                         qp[po:po + 64, :], start=True,
                                         stop=True, tile_position=(po, 0))
                        sT[(j, c)] = st
                    for j in range(2):
                        nc.scalar.activation(ck(expT[j], c), sT[(j, c)][:],
                                             AF.Exp)
                ds_sb = []
                for m in range(2):
                    dqp = dq_sb[m][:, kc * S:(kc + 1) * S]
                    dkp = dk_sb[m][:, kc * S:(kc + 1) * S]
                    dsb = [pool_h2.tile([128, NSC * S], BF, tag=f"ds{m}{j}",
                                        name=f"ds_sb{m}{j}", bufs=1)
                           for j in range(2)]
                    for c in range(NSC):
                        dsp = {}
                        for j, po in ((0, 0), (1, 64)):
                            ps = ps_big.tile([128, S], F32, tag="big",
                                             name="dsT")
                            nc.tensor.matmul(
                                ps[:], dkp[po:po + 64, c * 128:(c + 1) * 128],
                                qp[po:po + 64, :], start=True, stop=False,
                                tile_position=(po, 0))
                            nc.tensor.matmul(
                                ps[:], kp[po:po + 64, c * 128:(c + 1) * 128],
                                dqp[po:po + 64, :], start=False, stop=True,
                                tile_position=(po, 0))
                            dsp[j] = ps
                        for j in range(2):
                            if m == 0:
                                nc.scalar.copy(ck(dsb[j], c), dsp[j][:])
                            else:
                                nc.vector.tensor_copy(ck(dsb[j], c), dsp[j][:])
                    ds_sb.append(dsb)
                return expT, ds_sb

            def stage2(pi, expT, ds_sb):
                kc = pi
                sums = [ps_st.tile([128, S], F32, tag="st", name="sums")
                        for _ in range(2)]
                for j in range(2):
                    for c in range(NSC):
                        nc.tensor.matmul(sums[j][:], ones1[:], ck(expT[j], c),
                                         start=(c == 0), stop=(c == NSC - 1))
                rinv = []
                for j in range(2):
                    rf = f32tmp()
                    nc.vector.reciprocal_approx_fast(rf[:], sums[j][:])
                    rv = pool_h1.tile([128, S], BF, tag=f"rinv{j}",
                                      name=f"rinv{j}")
                    nc.scalar.copy(rv[:], rf[:])
                    rinv.append(rv)
                at = [pool_h1.tile([128, NSC * S], BF, tag=f"at{j}",
                                   name=f"at{j}") for j in range(2)]
                for j in range(2):
                    for c in range(NSC):
                        nc.vector.tensor_mul(ck(at[j], c), ck(expT[j], c),
                                             rinv[j][:])
                o_ps = ps_st.tile([128, S], F32, tag="st", name="o_ps")
                for c in range(NSC):
                    for j, po in ((0, 0), (1, 64)):
                        h = 2 * pi + j
                        vh = v_sb[:, c * D + h * HD: c * D + (h + 1) * HD]
                        nc.tensor.matmul(o_ps[po:po + 64, :], vh, ck(at[j], c),
                                         start=(c == 0), stop=(c == NSC - 1),
                                         tile_position=(0, po))
                osl = o_sb[:, kc * S:(kc + 1) * S]
                nc.scalar.copy(osl, o_ps[:])
                for m in range(2):
                    pt = [pool_h1.tile([128, NSC * S], BF, tag=f"pt{j}",
                                       name=f"pt{j}") for j in range(2)]
                    for j in range(2):
                        for c in range(NSC):
                            nc.vector.tensor_mul(ck(pt[j], c), ck(at[j], c),
                                                 ck(ds_sb[m][j], c))
                    c_ps = [ps_st.tile([128, S], F32, tag="st", name="c_ps2")
                            for _ in range(2)]
                    for j in range(2):
                        for c in range(NSC):
                            nc.tensor.matmul(c_ps[j][:], ones1[:],
                                             ck(pt[j], c), start=(c == 0),
                                             stop=(c == NSC - 1))
                    do_ps = ps_st.tile([128, S], F32, tag="st", name="do_ps")
                    for c in range(NSC):
                        for j, po in ((0, 0), (1, 64)):
                            h = 2 * pi + j
                            vh = v_sb[:, c * D + h * HD: c * D + (h + 1) * HD]
                            dvh = dv_sb[m][:, c * D + h * HD:
                                           c * D + (h + 1) * HD]
                            nc.tensor.matmul(do_ps[po:po + 64, :], vh,
                                             ck(pt[j], c), start=(c == 0),
                                             stop=False, tile_position=(0, po))
                            nc.tensor.matmul(do_ps[po:po + 64, :], dvh,
                                             ck(at[j], c), start=False,
                                             stop=(c == NSC - 1),
                                             tile_position=(0, po))
                    corr = pool_rot.tile([128, S], F32, tag="corr", name="corr", bufs=1)
                    for j, po in ((0, 0), (1, 64)):
                        nc.vector.tensor_mul(corr[po:po + 64, :],
                                             o_sb[po:po + 64,
                                                  kc * S:(kc + 1) * S],
                                             c_ps[j][po:po + 64, :])
                    nc.vector.tensor_sub(do_sb[m][:, kc * S:(kc + 1) * S],
                                         do_ps[:], corr[:])

            pend = None
            for pi in range(H // 2):
                args = stage1(pi)
                if pend is not None:
                    stage2(*pend)
                pend = (pi, *args)
            stage2(*pend)
        st_qkv.close()

        # =========== Phase D: proj + residual (spill xa f32 to DRAM) ========
        with tc.tile_pool(name="resid", bufs=1) as pool_res, \
             tc.tile_pool(name="wp2", bufs=1) as pool_wp2:
            # warm the sqrt table set while the PE is still busy, so LN2's
            # critical path doesn't pay the ACT table load
            warm = pool_rot.tile([128, 1], F32, tag="warm", name="warm")
            nc.scalar.activation(warm[:], epsb[:], AF.Sqrt, bias=epsb[:])
            for k in range(NK // 2, NK):
                wt = pool_wp2.tile([128, D], BF, tag=f"wp{k}", name=f"wp{k}")
                nc.sync.dma_start(wt[:], dram["wproj"][k])
                wproj_t.append(wt)
            # prefetch all residual inputs
            res_in = pool_res.tile([128, 3 * NK * S], F32, tag="resin",
                                   name="res_in")
            for si in range(3):
                for k in range(NK):
                    dst = res_in[:, (si * NK + k) * S:(si * NK + k + 1) * S]
                    if si == 0:
                        nc.scalar.dma_start(dst, dram["x_f32"][k])
                    else:
                        nc.sync.dma_start(dst, dram["t_f32"][si - 1, k])
            xabf = pool_lnio.tile([128, NK * S], BF, tag="xbf", name="xabf")
            dxabf = [pool_lnio.tile([128, NK * S], BF, tag=f"tbf{m}",
                                    name=f"dxabf{m}") for m in range(2)]
            psrcs = [o_sb, do_sb[0], do_sb[1]]
            pdsts = [xabf, dxabf[0], dxabf[1]]
            for mt in range(NK):
                for si in range(3):
                    ps = ps_big.tile([128, S], F32, tag="big", name="pj_ps")
                    for k in range(NK):
                        nc.tensor.matmul(ps[:],
                                         wproj_t[k][:, mt * 128:(mt + 1) * 128],
                                         ck(psrcs[si], k), start=(k == 0),
                                         stop=(k == NK - 1 and
                                               (si != 0 or not with_bias)))
                    if si == 0 and with_bias:
                        nc.tensor.matmul(
                            ps[:], bproj_sb[0:1, mt * 128:(mt + 1) * 128],
                            onesrow[:], start=False, stop=True)
                    xaf = f32tmp()
                    nc.vector.tensor_add(
                        xaf[:], res_in[:, (si * NK + mt) * S:
                                       (si * NK + mt + 1) * S], ps[:])
                    nc.sync.dma_start(xa_scr[si, mt], xaf[:])
                    nc.scalar.copy(ck(pdsts[si], mt), xaf[:])
        st_wp.close()
        st_o.close()

        # =========== Phase E: LN2 (primal first) ===========
        n2 = pool_lnout.tile([128, NK * S], BF, tag="n1", name="n2")
        dn2 = [pool_lnout.tile([128, NK * S], BF, tag=f"dn1{m}", name=f"dn2{m}")
               for m in range(2)]
        r2_bf = ln_primal(xabf, n2)
        ln_tangent(xabf, dxabf[0], n2, r2_bf, dn2[0])

        # =========== Phase F1: W1 + gelu, primal first ===========
        st_g = ExitStack()
        pool_g = st_g.enter_context(tc.tile_pool(name="gq", bufs=1))
        g_sb = pool_g.tile([128, NM1 * S], BF, tag="g", name="g_sb")
        q2_sb = [pool_g.tile([128, NM1 * S], BF, tag=f"q2{m}", name=f"q2_sb{m}")
                 for m in range(2)]
        with tc.tile_pool(name="dgp", bufs=1) as pool_dg:
            for half in range(2):
                dg_sb = pool_dg.tile([128, (NM1 // 2) * S], BF, tag="dg",
                                     name="dg_sb")
                with tc.tile_pool(name=f"w1h{half}", bufs=1) as pool_w1:
                    w1_t = []
                    for k in range(NK):
                        wt = pool_w1.tile([128, DFF // 2], BF, tag=f"w1{k}",
                                          name=f"w1_{half}_{k}")
                        nc.sync.dma_start(
                            wt[:], dram["w1"][k][:, half * (DFF // 2):
                                                 (half + 1) * (DFF // 2)])
                        w1_t.append(wt)
                    for mi in range(NM1 // 2):
                        mt = half * (NM1 // 2) + mi
                        ps = ps_big.tile([128, S], F32, tag="big", name="u_ps")
                        for k in range(NK):
                            nc.tensor.matmul(ps[:],
                                             w1_t[k][:, mi * 128:(mi + 1) * 128],
                                             ck(n2, k), start=(k == 0),
                                             stop=(k == NK - 1 and
                                                   not with_bias))
                        if with_bias:
                            nc.tensor.matmul(
                                ps[:], b1m_sb[0:1, mt * 128:(mt + 1) * 128],
                                onesrow[:], start=False, stop=True)
                        nc.scalar.activation(ck(g_sb, mt), ps[:], AF.Gelu)
                        nc.scalar.activation(ck(dg_sb, mi), ps[:],
                                             AF.Derivative_Gelu)
                    if half == 0:
                        ln_tangent(xabf, dxabf[1], n2, r2_bf, dn2[1])
                    for m in range(2):
                        for mi in range(NM1 // 2):
                            mt = half * (NM1 // 2) + mi
                            ps = ps_big.tile([128, S], F32, tag="big",
                                             name="ut_ps")
                            for k in range(NK):
                                nc.tensor.matmul(
                                    ps[:], w1_t[k][:, mi * 128:(mi + 1) * 128],
                                    ck(dn2[m], k), start=(k == 0),
                                    stop=(k == NK - 1))
                            nc.vector.tensor_mul(ck(q2_sb[m], mt),
                                                 ck(dg_sb, mi), ps[:])

        # =========== Phase F2: W2 + final residual ===========
        fsrcs = [g_sb, q2_sb[0], q2_sb[1]]
        with tc.tile_pool(name="w2p", bufs=1) as pool_w2, \
             tc.tile_pool(name="res2", bufs=1) as pool_res2:
            w2_t = []
            for k in range(NM1):
                wt = pool_w2.tile([128, D], BF, tag=f"w2{k}", name=f"w2_{k}")
                nc.scalar.dma_start(wt[:], dram["w2"][k])
                w2_t.append(wt)
            for si in range(3):
                for mt in range(NK):
                    res2 = pool_res2.tile([128, S], F32, tag="res2", bufs=6,
                                          name="res2")
                    nc.sync.dma_start(res2[:], xa_scr[si, mt])
                    ps = ps_big.tile([128, S], F32, tag="big", name="o2_ps")
                    for k in range(NM1):
                        nc.tensor.matmul(ps[:],
                                         w2_t[k][:, mt * 128:(mt + 1) * 128],
                                         ck(fsrcs[si], k), start=(k == 0),
                                         stop=(k == NM1 - 1))
                    ot = f32tmp()
                    nc.vector.tensor_add(ot[:], res2[:], ps[:])
                    nc.sync.dma_start(out_d[si, mt], ot[:])
        st_g.close()
        st_ab.close()
        st_n2.close()


def _prep_host(inputs):
    f32 = np.float32
    x = np.asarray(inputs["x"], f32)
    xt = np.asarray(inputs["x_tangent"], f32)
    g1 = np.asarray(inputs["g1"], f32); b1 = np.asarray(inputs["b1"], f32)
    g2 = np.asarray(inputs["g2"], f32); b2 = np.asarray(inputs["b2"], f32)
    Wqkv = np.asarray(inputs["Wqkv"], f32); Wproj = np.asarray(inputs["Wproj"], f32)
    W1 = np.asarray(inputs["W1"], f32); W2 = np.asarray(inputs["W2"], f32)
    bproj = np.asarray(inputs["bproj"], f32)
    bf1 = np.asarray(inputs["bf1"], f32); bf2 = np.asarray(inputs["bf2"], f32)

    bf16 = ml_dtypes.bfloat16
    Wqkv_f = g1[:, None] * Wqkv
    bqkv = b1 @ Wqkv
    W1_f = g2[:, None] * W1
    b1m = b2 @ W1 + bf1

    def tile_k(w, nk):
        return np.ascontiguousarray(w.reshape(nk, 128, -1))

    shared = {
        "wqkv": tile_k(Wqkv_f, NK).astype(bf16),
        "wproj": tile_k(Wproj, NK).astype(bf16),
        "w1": tile_k(W1_f, NK).astype(bf16),
        "w2": tile_k(W2, NM1).astype(bf16),
        "bqk": np.ascontiguousarray(bqkv[None, :2 * D]).astype(bf16),
        "bv": np.ascontiguousarray(bqkv[None, 2 * D:]).astype(bf16),
        "bproj": np.ascontiguousarray(bproj[None, :]).astype(bf16),
        "b1m": np.ascontiguousarray(b1m[None, :]).astype(bf16),
    }
    in_maps = []
    for core in range(N_CORES):
        b, mp = core // 2, core % 2
        xT = np.ascontiguousarray(x[b].T).reshape(NK, 128, S)
        tT = np.ascontiguousarray(
            xt[b, 2 * mp:2 * mp + 2].transpose(0, 2, 1)).reshape(2, NK, 128, S)
        im = dict(shared)
        im["x_f32"] = xT
        im["x_bf"] = xT.astype(bf16)
        im["t_f32"] = tT
        im["t_bf"] = tT.astype(bf16)
        in_maps.append(im)
    return in_maps, bf2


def kernel(**inputs):
    with_bias = not all(
        np.allclose(np.asarray(inputs[k]), 0.0)
        for k in ("b1", "bproj", "b2", "bf1"))
    key = ("nc", with_bias)
    if key not in _CACHE:
        _CACHE[key] = _build_program(with_bias)
    nc = _CACHE[key]
    in_maps, bf2 = _prep_host(inputs)
    res = run_bass_kernel_spmd(nc, in_maps, core_ids=list(range(N_CORES)),
                               **_RUN_KWARGS)
    _LAST_RES[0] = res
    out = np.zeros((B, S, D), np.float32)
    out_tan = np.zeros((B, M, S, D), np.float32)
    for core in range(N_CORES):
        b, mp = core // 2, core % 2
        o = res.results[core]["out"].reshape(3, D, S)
        if mp == 0:
            out[b] = o[0].T + bf2[None, :]
        out_tan[b, 2 * mp] = o[1].T
        out_tan[b, 2 * mp + 1] = o[2].T
    return out, out_tan


# revision 24
# speedup vs baseline: 1.2099x; 1.0058x over previous
# Trainium2 Bass kernel for a pre-norm transformer block with forward-mode JVP
# (jax.linearize) over M=4 tangent directions.
#
# Sharding: 8 cores; core c handles batch b=c//2 and tangents {2*(c%2), 2*(c%2)+1}.
# Each core computes the primal pass for its batch (even/odd core pairs do this
# redundantly; the even core's primal is used) plus 2 tangent JVP passes.
# No cross-core communication.
#
# On-chip layout is feature-major ([D, S] with features on partitions), so every
# linear layer is a plain accumulated matmul with no transposes. LayerNorm /
# softmax statistics (which reduce over partitions in this layout) are computed
# on the PE via ones-matmuls whose M=128 output broadcasts the column sums to
# all partitions. Softmax is computed without max-subtraction (scores are O(3)
# for this problem's data distribution). LN affine (g, b) is folded into the
# weights on the host; biases enter via K=1 ones-row matmuls; the final mlp
# bias bf2 is added on the host.
#
# Matmul inputs are bf16 (fp32 PSUM accumulation); the residual stream and LN /
# softmax statistics stay fp32 (residual adds read the fp32 inputs re-DMAed
# from DRAM; xa is spilled to DRAM scratch between the attention and MLP
# residual adds to stay under the SBUF budget).

import numpy as np
import ml_dtypes

import concourse.bass as bass
import concourse.tile as tile
from concourse import bacc, mybir
from concourse.bass_utils import run_bass_kernel_spmd

AF = mybir.ActivationFunctionType
BF = mybir.dt.bfloat16
F32 = mybir.dt.float32

B, S, D, H, M = 4, 512, 768, 12, 4
DFF = 4 * D
HD = D // H
EPS = 1e-6
SCALE = HD ** -0.5
NK = D // 128          # 6 feature chunks
NSC = S // 128         # 4 sequence chunks
NM1 = DFF // 128       # 24
N_CORES = 8

_CACHE = {}
_RUN_KWARGS = {}   # test harness can set {"trace": True}
_LAST_RES = [None]


def _build_program(with_bias):
    nc = bacc.Bacc("TRN2", target_bir_lowering=False, debug=False,
                   num_devices=N_CORES)

    dram = {}
    def din(name, shape, dt):
        dram[name] = nc.dram_tensor(name, shape, dt, kind="ExternalInput").ap()
    din("x_f32", [NK, 128, S], F32)
    din("x_bf", [NK, 128, S], BF)
    din("t_f32", [2, NK, 128, S], F32)
    din("t_bf", [2, NK, 128, S], BF)
    din("wqkv", [NK, 128, 3 * D], BF)
    din("wproj", [NK, 128, D], BF)
    din("w1", [NK, 128, DFF], BF)
    din("w2", [NM1, 128, D], BF)
    din("bqk", [1, 2 * D], BF)
    din("bv", [1, D], BF)
    din("bproj", [1, D], BF)
    din("b1m", [1, DFF], BF)
    out_d = nc.dram_tensor("out", [3, NK, 128, S], F32, kind="ExternalOutput").ap()
    xa_scr = nc.dram_tensor("xa_scr", [3, NK, 128, S], F32).ap()
    xab_scr = nc.dram_tensor("xab_scr", [3, NK, 128, S], BF).ap()

    with tile.TileContext(nc) as tc:
        _emit(nc, tc, dram, out_d, xa_scr, xab_scr, with_bias)
    nc.compile()
    return nc


def _emit(nc, tc, dram, out_d, xa_scr, xab_scr, with_bias):
    from contextlib import ExitStack
    ctx = ExitStack()
    with ctx:
        pool_const = ctx.enter_context(tc.tile_pool(name="const", bufs=1))
        pool_rot = ctx.enter_context(tc.tile_pool(name="rot", bufs=2))
        pool_stat = ctx.enter_context(tc.tile_pool(name="stat", bufs=1))
        ps_big = ctx.enter_context(tc.tile_pool(name="psbig", bufs=5, space="PSUM"))
        ps_st = ctx.enter_context(tc.tile_pool(name="psst", bufs=3, space="PSUM"))

        # ---- constants ----
        ones1 = pool_const.tile([128, 128], BF, tag="ones1")
        nc.gpsimd.memset(ones1[:], 1.0)
        onesd = pool_const.tile([128, 128], BF, tag="onesd")
        nc.gpsimd.memset(onesd[:], 1.0 / D)
        onesrow = pool_const.tile([1, S], BF, tag="onesrow")
        nc.gpsimd.memset(onesrow[:], 1.0)
        epsb = pool_const.tile([128, 1], F32, tag="epsb")
        nc.gpsimd.memset(epsb[:], EPS)
        bqk_sb = pool_const.tile([1, 2 * D], BF, tag="bqk")
        nc.sync.dma_start(bqk_sb[:], dram["bqk"][:])
        bv_sb = pool_const.tile([1, D], BF, tag="bv")
        nc.sync.dma_start(bv_sb[:], dram["bv"][:])
        bproj_sb = pool_const.tile([1, D], BF, tag="bproj")
        nc.sync.dma_start(bproj_sb[:], dram["bproj"][:])
        b1m_sb = pool_const.tile([1, DFF], BF, tag="b1m")
        nc.sync.dma_start(b1m_sb[:], dram["b1m"][:])

        warm0 = pool_const.tile([128, 1], F32, tag="warm0")
        nc.scalar.activation(warm0[:], epsb[:], AF.Sqrt, bias=epsb[:])

        def ck(t, k):
            return t[:, k * S:(k + 1) * S]

        def f32tmp():
            return pool_rot.tile([128, S], F32, tag="f32tmp", bufs=3,
                                 name="f32tmp")

        # ---- LayerNorm: primal part (stats via PE colsum-broadcast) ----
        def ln_primal(in_bf, n_bf):
            mu_ps = ps_st.tile([128, S], F32, tag="st", name="mu_ps")
            s2_ps = ps_st.tile([128, S], F32, tag="st", name="s2_ps")
            for k in range(NK):
                sq = pool_rot.tile([128, S], BF, tag="sq", name="sq")
                nc.vector.tensor_mul(sq[:], ck(in_bf, k), ck(in_bf, k))
                nc.tensor.matmul(mu_ps[:], onesd[:], ck(in_bf, k),
                                 start=(k == 0), stop=(k == NK - 1))
                nc.tensor.matmul(s2_ps[:], onesd[:], sq[:],
                                 start=(k == 0), stop=(k == NK - 1))
            mu_f = pool_stat.tile([128, S], F32, tag="lnmu", name="mu_f")
            nc.scalar.copy(mu_f[:], mu_ps[:])
            mu2 = pool_rot.tile([128, S], BF, tag="sq", name="mu2")
            nc.vector.tensor_mul(mu2[:], mu_f[:], mu_f[:])
            var = f32tmp()
            nc.vector.tensor_sub(var[:], s2_ps[:], mu2[:])
            sd = f32tmp()
            nc.scalar.activation(sd[:], var[:], AF.Sqrt, bias=epsb[:])
            r_f = pool_stat.tile([128, S], F32, tag="lnr", name="r_f")
            nc.vector.reciprocal_approx_fast(r_f[:], sd[:])
            for k in range(NK):
                cen = pool_rot.tile([128, S], F32, tag="cen", name="cen")
                nc.vector.tensor_sub(cen[:], ck(in_bf, k), mu_f[:])
                nc.vector.tensor_mul(ck(n_bf, k), cen[:], r_f[:])
            return r_f

        # ---- LayerNorm: one tangent's JVP ----
        def ln_tangent(in_bf, tan_bf, n_bf, r_bf, dn_bf):  # r_bf is f32 now
            mt_ps = ps_st.tile([128, S], F32, tag="st", name="mt_ps")
            c_ps = ps_st.tile([128, S], F32, tag="st", name="c_ps")
            for k in range(NK):
                p = pool_rot.tile([128, S], BF, tag="p", name="p")
                nc.vector.tensor_mul(p[:], ck(n_bf, k), ck(tan_bf, k))
                nc.tensor.matmul(mt_ps[:], onesd[:], ck(tan_bf, k),
                                 start=(k == 0), stop=(k == NK - 1))
                nc.tensor.matmul(c_ps[:], onesd[:], p[:],
                                 start=(k == 0), stop=(k == NK - 1))
            ctr = pool_rot.tile([128, S], BF, tag="ctr", name="ctr")
            nc.vector.tensor_mul(ctr[:], c_ps[:], r_bf[:])
            mt_f = pool_rot.tile([128, S], F32, tag="f32tmp", bufs=3, name="mt_f")
            nc.scalar.copy(mt_f[:], mt_ps[:])
            for k in range(NK):
                b_ = pool_rot.tile([128, S], F32, tag="cen", name="b_")
                nc.vector.tensor_sub(b_[:], ck(tan_bf, k), mt_f[:])
                e_ = pool_rot.tile([128, S], F32, tag="e", name="e_")
                nc.vector.tensor_mul(e_[:], b_[:], r_bf[:])
                f_ = pool_rot.tile([128, S], BF, tag="f", name="f_")
                nc.vector.tensor_mul(f_[:], ck(n_bf, k), ctr[:])
                nc.vector.tensor_sub(ck(dn_bf, k), e_[:], f_[:])

        # LN input/output pools outlive the o_sb pool (LIFO nesting); the
        # LN2 tensors reuse the LN1 tags (slot reuse after LN1 consumers end).
        st_n2 = ExitStack()
        pool_lnout = st_n2.enter_context(tc.tile_pool(name="lnout", bufs=1))
        st_ab = ExitStack()
        pool_lnio = st_ab.enter_context(tc.tile_pool(name="lnio", bufs=1))

        st_o = ExitStack()
        pool_o = st_o.enter_context(tc.tile_pool(name="osb", bufs=1))
        o_sb = pool_o.tile([128, NK * S], BF, tag="o", name="o_sb")
        do_sb = [pool_o.tile([128, NK * S], BF, tag=f"do{m}", name=f"do_sb{m}")
                 for m in range(2)]

        st_wp = ExitStack()
        pool_wp = st_wp.enter_context(tc.tile_pool(name="wp", bufs=1))
        wproj_t = []
        for k in range(NK // 2):
            wt = pool_wp.tile([128, D], BF, tag=f"wp{k}", name=f"wp{k}")
            nc.sync.dma_start(wt[:], dram["wproj"][k])
            wproj_t.append(wt)

        st_qkv = ExitStack()
        pool_qkv = st_qkv.enter_context(tc.tile_pool(name="qkv", bufs=1))
        q_sb = pool_qkv.tile([128, NK * S], BF, tag="q", name="q_sb")
        k_sb = pool_qkv.tile([128, NK * S], BF, tag="k", name="k_sb")
        dq_sb = [pool_qkv.tile([128, NK * S], BF, tag=f"dq{m}", name=f"dq_sb{m}")
                 for m in range(2)]
        dk_sb = [pool_qkv.tile([128, NK * S], BF, tag=f"dk{m}", name=f"dk_sb{m}")
                 for m in range(2)]
        v_sb = pool_qkv.tile([128, NSC * D], BF, tag="v", name="v_sb")
        dv_sb = [pool_qkv.tile([128, NSC * D], BF, tag=f"dv{m}", name=f"dv_sb{m}")
                 for m in range(2)]

        # =========== Phases A+B: LN1 and QKV, primal first ===========
        with tc.tile_pool(name="wq", bufs=1) as pool_wq:
            n1 = pool_lnout.tile([128, NK * S], BF, tag="n1", name="n1")
            dn1 = [pool_lnout.tile([128, NK * S], BF, tag=f"dn1{m}",
                                   name=f"dn1{m}") for m in range(2)]
            xbf = pool_lnio.tile([128, NK * S], BF, tag="xbf", name="xbf")
            tbf = [pool_lnio.tile([128, NK * S], BF, tag=f"tbf{m}",
                                  name=f"tbf{m}") for m in range(2)]
            for k in range(NK):
                nc.scalar.dma_start(ck(xbf, k), dram["x_bf"][k])
            wqkv_t = []
            for k in range(NK):
                wt = pool_wq.tile([128, 3 * D], BF, tag=f"wqkv{k}",
                                  name=f"wqkv{k}")
                nc.scalar.dma_start(wt[:], dram["wqkv"][k])
                wqkv_t.append(wt)
            for k in range(NK):
                for m in range(2):
                    nc.sync.dma_start(ck(tbf[m], k), dram["t_bf"][m, k])

            r1_bf = ln_primal(xbf, n1)

            def qk_pass2(src, qd, kd, with_bias):
                for mt in range(12):
                    ps = ps_big.tile([128, S], F32, tag="big", name="qkv_ps")
                    for k in range(NK):
                        nc.tensor.matmul(ps[:],
                                         wqkv_t[k][:, mt * 128:(mt + 1) * 128],
                                         ck(src, k), start=(k == 0),
                                         stop=(k == NK - 1 and not with_bias))
                    if with_bias:
                        nc.tensor.matmul(ps[:],
                                         bqk_sb[0:1, mt * 128:(mt + 1) * 128],
                                         onesrow[:], start=False, stop=True)
                    if mt < 6:
                        nc.scalar.mul(qd[:, mt * S:(mt + 1) * S], ps[:], SCALE)
                    else:
                        nc.scalar.copy(kd[:, (mt - 6) * S:(mt - 5) * S], ps[:])

            def v_pass(src, vd, with_bias):
                for sc in range(NSC):
                    for g2 in range(2):
                        wv_col = 2 * D + g2 * 384
                        ps = ps_big.tile([128, 384], F32, tag="big", name="v_ps")
                        for k in range(NK):
                            lhs = src[:, k * S + sc * 128: k * S + (sc + 1) * 128]
                            nc.tensor.matmul(ps[:], lhs,
                                             wqkv_t[k][:, wv_col:wv_col + 384],
                                             start=(k == 0),
                                             stop=(k == NK - 1 and not with_bias))
                        if with_bias:
                            nc.tensor.matmul(ps[:], ones1[0:1, :],
                                             bv_sb[0:1, g2 * 384:(g2 + 1) * 384],
                                             start=False, stop=True)
                        col = sc * D + g2 * 384
                        nc.scalar.copy(vd[:, col:col + 384], ps[:])

            qk_pass2(n1, q_sb, k_sb, with_bias)
            v_pass(n1, v_sb, with_bias)
            for m in range(2):
                ln_tangent(xbf, tbf[m], n1, r1_bf, dn1[m])
            for m in range(2):
                qk_pass2(dn1[m], dq_sb[m], dk_sb[m], False)
                v_pass(dn1[m], dv_sb[m], False)

        # =========== Phase C: attention, head pairs ===========
        # Heads are processed in pairs (2i, 2i+1) sharing one feature chunk:
        # the K=64 score matmuls of the two heads run concurrently in the PE
        # via row tile_position (0,0)/(64,0); the M=64 attention-value matmuls
        # share one PSUM bank via column tile_position (0,0)/(0,64).
        # Two-stage software pipeline as before, one pair per iteration.
        with tc.tile_pool(name="head2", bufs=2) as pool_h2, \
             tc.tile_pool(name="head1", bufs=1) as pool_h1:
            def stage1(pi):
                kc = pi
                qp = q_sb[:, kc * S:(kc + 1) * S]
                kp = k_sb[:, kc * S:(kc + 1) * S]
                sT = {}
                expT = [pool_h2.tile([128, NSC * S], BF, tag=f"exp{j}",
                                     name=f"expT{j}") for j in range(2)]
                for c in range(NSC):
                    for j, po in ((0, 0), (1, 64)):
                        st = ps_big.tile([128, S], F32, tag="big", name="sT")
                        nc.tensor.matmul(st[:],
                                         kp[po:po + 64, c * 128:(c + 1) * 128],
                                         qp[po:po + 64, :], start=True,
                                         stop=True, tile_position=(po, 0))
                        sT[(j, c)] = st
                    for j in range(2):
                        nc.scalar.activation(ck(expT[j], c), sT[(j, c)][:],
                                             AF.Exp)
                ds_sb = []
                for m in range(2):
                    dqp = dq_sb[m][:, kc * S:(kc + 1) * S]
                    dkp = dk_sb[m][:, kc * S:(kc + 1) * S]
                    dsb = [pool_h2.tile([128, NSC * S], BF, tag=f"ds{m}{j}",
                                        name=f"ds_sb{m}{j}", bufs=1)
                           for j in range(2)]
                    for c in range(NSC):
                        dsp = {}
                        for j, po in ((0, 0), (1, 64)):
                            ps = ps_big.tile([128, S], F32, tag="big",
                                             name="dsT")
                            nc.tensor.matmul(
                                ps[:], dkp[po:po + 64, c * 128:(c + 1) * 128],
                                qp[po:po + 64, :], start=True, stop=False,
                                tile_position=(po, 0))
                            nc.tensor.matmul(
                                ps[:], kp[po:po + 64, c * 128:(c + 1) * 128],
                                dqp[po:po + 64, :], start=False, stop=True,
                                tile_position=(po, 0))
                            dsp[j] = ps
                        for j in range(2):
                            if m == 0:
                                nc.scalar.copy(ck(dsb[j], c), dsp[j][:])
                            else:
                                nc.vector.tensor_copy(ck(dsb[j], c), dsp[j][:])
                    ds_sb.append(dsb)
                return expT, ds_sb

            def stage2(pi, expT, ds_sb):
                kc = pi
                sums = [ps_st.tile([128, S], F32, tag="st", name="sums")
                        for _ in range(2)]
                for j in range(2):
                    for c in range(NSC):
                        nc.tensor.matmul(sums[j][:], ones1[:], ck(expT[j], c),
                                         start=(c == 0), stop=(c == NSC - 1))
                rinv = []
                for j in range(2):
                    rf = f32tmp()
                    nc.vector.reciprocal_approx_fast(rf[:], sums[j][:])
                    rv = pool_h1.tile([128, S], BF, tag=f"rinv{j}",
                                      name=f"rinv{j}")
                    nc.scalar.copy(rv[:], rf[:])
                    rinv.append(rv)
                at = [pool_h1.tile([128, NSC * S], BF, tag=f"at{j}",
                                   name=f"at{j}") for j in range(2)]
                for j in range(2):
                    for c in range(NSC):
                        nc.vector.tensor_mul(ck(at[j], c), ck(expT[j], c),
                                             rinv[j][:])
                o_ps = ps_st.tile([128, S], F32, tag="st", name="o_ps")
                for c in range(NSC):
                    for j, po in ((0, 0), (1, 64)):
                        h = 2 * pi + j
                        vh = v_sb[:, c * D + h * HD: c * D + (h + 1) * HD]
                        nc.tensor.matmul(o_ps[po:po + 64, :], vh, ck(at[j], c),
                                         start=(c == 0), stop=(c == NSC - 1),
                                         tile_position=(0, po))
                osl = o_sb[:, kc * S:(kc + 1) * S]
                nc.scalar.copy(osl, o_ps[:])
                for m in range(2):
                    pt = [pool_h1.tile([128, NSC * S], BF, tag=f"pt{j}",
                                       name=f"pt{j}") for j in range(2)]
                    for j in range(2):
                        for c in range(NSC):
                            nc.vector.tensor_mul(ck(pt[j], c), ck(at[j], c),
                                                 ck(ds_sb[m][j], c))
                    c_ps = [ps_st.tile([128, S], F32, tag="st", name="c_ps2")
                            for _ in range(2)]
                    for j in range(2):
                        for c in range(NSC):
                            nc.tensor.matmul(c_ps[j][:], ones1[:],
                                             ck(pt[j], c), start=(c == 0),
                                             stop=(c == NSC - 1))
                    do_ps = ps_st.tile([128, S], F32, tag="st", name="do_ps")
                    for c in range(NSC):
                        for j, po in ((0, 0), (1, 64)):
                            h = 2 * pi + j
                            vh = v_sb[:, c * D + h * HD: c * D + (h + 1) * HD]
                            dvh = dv_sb[m][:, c * D + h * HD:
                                           c * D + (h + 1) * HD]
                            nc.tensor.matmul(do_ps[po:po + 64, :], vh,
                                             ck(pt[j], c), start=(c == 0),
                                             stop=False, tile_position=(0, po))
                            nc.tensor.matmul(do_ps[po:po + 64, :], dvh,
                                             ck(at[j], c), start=False,
                                             stop=(c == NSC - 1),
                                             tile_position=(0, po))
                    corr = pool_rot.tile([128, S], F32, tag="corr", name="corr", bufs=1)
                    for j, po in ((0, 0), (1, 64)):
                        nc.vector.tensor_mul(corr[po:po + 64, :],
                                             o_sb[po:po + 64,
                                                  kc * S:(kc + 1) * S],
                                             c_ps[j][po:po + 64, :])
                    nc.vector.tensor_sub(do_sb[m][:, kc * S:(kc + 1) * S],
                                         do_ps[:], corr[:])

            pend = None
            for pi in range(H // 2):
                args = stage1(pi)
                if pend is not None:
                    stage2(*pend)
                pend = (pi, *args)
            stage2(*pend)
        st_qkv.close()

        # =========== Phase D: proj + residual (spill xa f32 to DRAM) ========
        with tc.tile_pool(name="resid", bufs=1) as pool_res, \
             tc.tile_pool(name="wp2", bufs=1) as pool_wp2:
            # warm the sqrt table set while the PE is still busy, so LN2's
            # critical path doesn't pay the ACT table load
            warm = pool_rot.tile([128, 1], F32, tag="warm", name="warm")
            nc.scalar.activation(warm[:], epsb[:], AF.Sqrt, bias=epsb[:])
            for k in range(NK // 2, NK):
                wt = pool_wp2.tile([128, D], BF, tag=f"wp{k}", name=f"wp{k}")
                nc.sync.dma_start(wt[:], dram["wproj"][k])
                wproj_t.append(wt)
            # prefetch all residual inputs
            res_in = pool_res.tile([128, 3 * NK * S], F32, tag="resin",
                                   name="res_in")
            for si in range(3):
                for k in range(NK):
                    dst = res_in[:, (si * NK + k) * S:(si * NK + k + 1) * S]
                    if si == 0:
                        nc.scalar.dma_start(dst, dram["x_f32"][k])
                    else:
                        nc.sync.dma_start(dst, dram["t_f32"][si - 1, k])
            xabf = pool_lnio.tile([128, NK * S], BF, tag="xbf", name="xabf")
            dxabf = [pool_lnio.tile([128, NK * S], BF, tag=f"tbf{m}",
                                    name=f"dxabf{m}") for m in range(2)]
            psrcs = [o_sb, do_sb[0], do_sb[1]]
            pdsts = [xabf, dxabf[0], dxabf[1]]
            for mt in range(NK):
                for si in range(3):
                    ps = ps_big.tile([128, S], F32, tag="big", name="pj_ps")
                    for k in range(NK):
                        nc.tensor.matmul(ps[:],
                                         wproj_t[k][:, mt * 128:(mt + 1) * 128],
                                         ck(psrcs[si], k), start=(k == 0),
                                         stop=(k == NK - 1 and
                                               (si != 0 or not with_bias)))
                    if si == 0 and with_bias:
                        nc.tensor.matmul(
                            ps[:], bproj_sb[0:1, mt * 128:(mt + 1) * 128],
                            onesrow[:], start=False, stop=True)
                    xaf = f32tmp()
                    nc.vector.tensor_add(
                        xaf[:], res_in[:, (si * NK + mt) * S:
                                       (si * NK + mt + 1) * S], ps[:])
                    nc.sync.dma_start(xa_scr[si, mt], xaf[:])
                    nc.scalar.copy(ck(pdsts[si], mt), xaf[:])
        st_wp.close()
        st_o.close()

        # =========== Phase E: LN2 (primal first) ===========
        n2 = pool_lnout.tile([128, NK * S], BF, tag="n1", name="n2")
        dn2 = [pool_lnout.tile([128, NK * S], BF, tag=f"dn1{m}", name=f"dn2{m}")
               for m in range(2)]
        r2_bf = ln_primal(xabf, n2)
        ln_tangent(xabf, dxabf[0], n2, r2_bf, dn2[0])

        # =========== Phase F1: W1 + gelu, primal first ===========
        st_g = ExitStack()
        pool_g = st_g.enter_context(tc.tile_pool(name="gq", bufs=1))
        g_sb = pool_g.tile([128, NM1 * S], BF, tag="g", name="g_sb")
        q2_sb = [pool_g.tile([128, NM1 * S], BF, tag=f"q2{m}", name=f"q2_sb{m}")
                 for m in range(2)]
        with tc.tile_pool(name="dgp", bufs=1) as pool_dg:
            for half in range(2):
                dg_sb = pool_dg.tile([128, (NM1 // 2) * S], BF, tag="dg",
                                     name="dg_sb")
                with tc.tile_pool(name=f"w1h{half}", bufs=1) as pool_w1:
                    w1_t = []
                    for k in range(NK):
                        wt = pool_w1.tile([128, DFF // 2], BF, tag=f"w1{k}",
                                          name=f"w1_{half}_{k}")
                        nc.sync.dma_start(
                            wt[:], dram["w1"][k][:, half * (DFF // 2):
                                                 (half + 1) * (DFF // 2)])
                        w1_t.append(wt)
                    for mi in range(NM1 // 2):
                        mt = half * (NM1 // 2) + mi
                        ps = ps_big.tile([128, S], F32, tag="big", name="u_ps")
                        for k in range(NK):
                            nc.tensor.matmul(ps[:],
                                             w1_t[k][:, mi * 128:(mi + 1) * 128],
                                             ck(n2, k), start=(k == 0),
                                             stop=(k == NK - 1 and
                                                   not with_bias))
                        if with_bias:
                            nc.tensor.matmul(
                                ps[:], b1m_sb[0:1, mt * 128:(mt + 1) * 128],
                                onesrow[:], start=False, stop=True)
                        nc.scalar.activation(ck(g_sb, mt), ps[:], AF.Gelu)
                        nc.scalar.activation(ck(dg_sb, mi), ps[:],
                                             AF.Derivative_Gelu)
                    if half == 0:
                        ln_tangent(xabf, dxabf[1], n2, r2_bf, dn2[1])
                    for m in range(2):
                        for mi in range(NM1 // 2):
                            mt = half * (NM1 // 2) + mi
                            ps = ps_big.tile([128, S], F32, tag="big",
                                             name="ut_ps")
                            for k in range(NK):
                                nc.tensor.matmul(
                                    ps[:], w1_t[k][:, mi * 128:(mi + 1) * 128],
                                    ck(dn2[m], k), start=(k == 0),
                                    stop=(k == NK - 1))
                            nc.vector.tensor_mul(ck(q2_sb[m], mt),
                                                 ck(dg_sb, mi), ps[:])

        # =========== Phase F2: W2 + final residual ===========
        fsrcs = [g_sb, q2_sb[0], q2_sb[1]]
        with tc.tile_pool(name="w2p", bufs=1) as pool_w2, \
             tc.tile_pool(name="res2", bufs=1) as pool_res2:
            w2_t = []
            for k in range(NM1):
                wt = pool_w2.tile([128, D], BF, tag=f"w2{k}", name=f"w2_{k}")
                nc.scalar.dma_start(wt[:], dram["w2"][k])
                w2_t.append(wt)
            for si in range(3):
                for mt in range(NK):
                    res2 = pool_res2.tile([128, S], F32, tag="res2", bufs=6,
                                          name="res2")
                    nc.sync.dma_start(res2[:], xa_scr[si, mt])
                    ps = ps_big.tile([128, S], F32, tag="big", name="o2_ps")
                    for k in range(NM1):
                        nc.tensor.matmul(ps[:],
                                         w2_t[k][:, mt * 128:(mt + 1) * 128],
                                         ck(fsrcs[si], k), start=(k == 0),
                                         stop=(k == NM1 - 1))
                    ot = f32tmp()
                    nc.vector.tensor_add(ot[:], res2[:], ps[:])
                    nc.sync.dma_start(out_d[si, mt], ot[:])
        st_g.close()
        st_ab.close()
        st_n2.close()


def _prep_host(inputs):
    f32 = np.float32
    x = np.asarray(inputs["x"], f32)
    xt = np.asarray(inputs["x_tangent"], f32)
    g1 = np.asarray(inputs["g1"], f32); b1 = np.asarray(inputs["b1"], f32)
    g2 = np.asarray(inputs["g2"], f32); b2 = np.asarray(inputs["b2"], f32)
    Wqkv = np.asarray(inputs["Wqkv"], f32); Wproj = np.asarray(inputs["Wproj"], f32)
    W1 = np.asarray(inputs["W1"], f32); W2 = np.asarray(inputs["W2"], f32)
    bproj = np.asarray(inputs["bproj"], f32)
    bf1 = np.asarray(inputs["bf1"], f32); bf2 = np.asarray(inputs["bf2"], f32)

    bf16 = ml_dtypes.bfloat16
    Wqkv_f = g1[:, None] * Wqkv
    bqkv = b1 @ Wqkv
    W1_f = g2[:, None] * W1
    b1m = b2 @ W1 + bf1

    def tile_k(w, nk):
        return np.ascontiguousarray(w.reshape(nk, 128, -1))

    shared = {
        "wqkv": tile_k(Wqkv_f, NK).astype(bf16),
        "wproj": tile_k(Wproj, NK).astype(bf16),
        "w1": tile_k(W1_f, NK).astype(bf16),
        "w2": tile_k(W2, NM1).astype(bf16),
        "bqk": np.ascontiguousarray(bqkv[None, :2 * D]).astype(bf16),
        "bv": np.ascontiguousarray(bqkv[None, 2 * D:]).astype(bf16),
        "bproj": np.ascontiguousarray(bproj[None, :]).astype(bf16),
        "b1m": np.ascontiguousarray(b1m[None, :]).astype(bf16),
    }
    in_maps = []
    for core in range(N_CORES):
        b, mp = core // 2, core % 2
        xT = np.ascontiguousarray(x[b].T).reshape(NK, 128, S)
        tT = np.ascontiguousarray(
            xt[b, 2 * mp:2 * mp + 2].transpose(0, 2, 1)).reshape(2, NK, 128, S)
        im = dict(shared)
        im["x_f32"] = xT
        im["x_bf"] = xT.astype(bf16)
        im["t_f32"] = tT
        im["t_bf"] = tT.astype(bf16)
        in_maps.append(im)
    return in_maps, bf2


def kernel(**inputs):
    with_bias = not all(
        np.allclose(np.asarray(inputs[k]), 0.0)
        for k in ("b1", "bproj", "b2", "bf1"))
    key = ("nc", with_bias)
    if key not in _CACHE:
        _CACHE[key] = _build_program(with_bias)
    nc = _CACHE[key]
    in_maps, bf2 = _prep_host(inputs)
    res = run_bass_kernel_spmd(nc, in_maps, core_ids=list(range(N_CORES)),
                               **_RUN_KWARGS)
    _LAST_RES[0] = res
    out = np.zeros((B, S, D), np.float32)
    out_tan = np.zeros((B, M, S, D), np.float32)
    for core in range(N_CORES):
        b, mp = core // 2, core % 2
        o = res.results[core]["out"].reshape(3, D, S)
        if mp == 0:
            out[b] = o[0].T + bf2[None, :]
        out_tan[b, 2 * mp] = o[1].T
        out_tan[b, 2 * mp + 1] = o[2].T
    return out, out_tan


# revision 25
# speedup vs baseline: 1.2134x; 1.0029x over previous
# Trainium2 Bass kernel for a pre-norm transformer block with forward-mode JVP
# (jax.linearize) over M=4 tangent directions.
#
# Sharding: 8 cores; core c handles batch b=c//2 and tangents {2*(c%2), 2*(c%2)+1}.
# Each core computes the primal pass for its batch (even/odd core pairs do this
# redundantly; the even core's primal is used) plus 2 tangent JVP passes.
# No cross-core communication.
#
# On-chip layout is feature-major ([D, S] with features on partitions), so every
# linear layer is a plain accumulated matmul with no transposes. LayerNorm /
# softmax statistics (which reduce over partitions in this layout) are computed
# on the PE via ones-matmuls whose M=128 output broadcasts the column sums to
# all partitions. Softmax is computed without max-subtraction (scores are O(3)
# for this problem's data distribution). LN affine (g, b) is folded into the
# weights on the host; biases enter via K=1 ones-row matmuls; the final mlp
# bias bf2 is added on the host.
#
# Matmul inputs are bf16 (fp32 PSUM accumulation); the residual stream and LN /
# softmax statistics stay fp32 (residual adds read the fp32 inputs re-DMAed
# from DRAM; xa is spilled to DRAM scratch between the attention and MLP
# residual adds to stay under the SBUF budget).

import numpy as np
import ml_dtypes

import concourse.bass as bass
import concourse.tile as tile
from concourse import bacc, mybir
from concourse.bass_utils import run_bass_kernel_spmd

AF = mybir.ActivationFunctionType
BF = mybir.dt.bfloat16
F32 = mybir.dt.float32

B, S, D, H, M = 4, 512, 768, 12, 4
DFF = 4 * D
HD = D // H
EPS = 1e-6
SCALE = HD ** -0.5
NK = D // 128          # 6 feature chunks
NSC = S // 128         # 4 sequence chunks
NM1 = DFF // 128       # 24
N_CORES = 8

_CACHE = {}
_RUN_KWARGS = {}   # test harness can set {"trace": True}
_LAST_RES = [None]


def _build_program(with_bias):
    nc = bacc.Bacc("TRN2", target_bir_lowering=False, debug=False,
                   num_devices=N_CORES)

    dram = {}
    def din(name, shape, dt):
        dram[name] = nc.dram_tensor(name, shape, dt, kind="ExternalInput").ap()
    din("x_f32", [NK, 128, S], F32)
    din("x_bf", [NK, 128, S], BF)
    din("t_f32", [2, NK, 128, S], F32)
    din("t_bf", [2, NK, 128, S], BF)
    din("wqkv", [NK, 128, 3 * D], BF)
    din("wproj", [NK, 128, D], BF)
    din("w1", [NK, 128, DFF], BF)
    din("w2", [NM1, 128, D], BF)
    din("bqk", [1, 2 * D], BF)
    din("bv", [1, D], BF)
    din("bproj", [1, D], BF)
    din("b1m", [1, DFF], BF)
    out_d = nc.dram_tensor("out", [3, NK, 128, S], F32, kind="ExternalOutput").ap()
    xa_scr = nc.dram_tensor("xa_scr", [3, NK, 128, S], F32).ap()
    xab_scr = nc.dram_tensor("xab_scr", [3, NK, 128, S], BF).ap()

    with tile.TileContext(nc) as tc:
        _emit(nc, tc, dram, out_d, xa_scr, xab_scr, with_bias)
    nc.compile()
    return nc


def _emit(nc, tc, dram, out_d, xa_scr, xab_scr, with_bias):
    from contextlib import ExitStack
    ctx = ExitStack()
    with ctx:
        pool_const = ctx.enter_context(tc.tile_pool(name="const", bufs=1))
        pool_rot = ctx.enter_context(tc.tile_pool(name="rot", bufs=2))
        pool_stat = ctx.enter_context(tc.tile_pool(name="stat", bufs=1))
        ps_big = ctx.enter_context(tc.tile_pool(name="psbig", bufs=5, space="PSUM"))
        ps_st = ctx.enter_context(tc.tile_pool(name="psst", bufs=3, space="PSUM"))

        # ---- constants ----
        ones1 = pool_const.tile([128, 128], BF, tag="ones1")
        nc.gpsimd.memset(ones1[:], 1.0)
        onesd = pool_const.tile([128, 128], BF, tag="onesd")
        nc.gpsimd.memset(onesd[:], 1.0 / D)
        onesrow = pool_const.tile([1, S], BF, tag="onesrow")
        nc.gpsimd.memset(onesrow[:], 1.0)
        epsb = pool_const.tile([128, 1], F32, tag="epsb")
        nc.gpsimd.memset(epsb[:], EPS)
        bqk_sb = pool_const.tile([1, 2 * D], BF, tag="bqk")
        nc.sync.dma_start(bqk_sb[:], dram["bqk"][:])
        bv_sb = pool_const.tile([1, D], BF, tag="bv")
        nc.sync.dma_start(bv_sb[:], dram["bv"][:])
        bproj_sb = pool_const.tile([1, D], BF, tag="bproj")
        nc.sync.dma_start(bproj_sb[:], dram["bproj"][:])
        b1m_sb = pool_const.tile([1, DFF], BF, tag="b1m")
        nc.sync.dma_start(b1m_sb[:], dram["b1m"][:])

        warm0 = pool_const.tile([128, 1], F32, tag="warm0")
        nc.scalar.activation(warm0[:], epsb[:], AF.Sqrt, bias=epsb[:])

        def ck(t, k):
            return t[:, k * S:(k + 1) * S]

        def f32tmp():
            return pool_rot.tile([128, S], F32, tag="f32tmp", bufs=4,
                                 name="f32tmp")

        # ---- LayerNorm: primal part (stats via PE colsum-broadcast) ----
        def ln_primal(in_bf, n_bf):
            mu_ps = ps_st.tile([128, S], F32, tag="st", name="mu_ps")
            s2_ps = ps_st.tile([128, S], F32, tag="st", name="s2_ps")
            for k in range(NK):
                sq = pool_rot.tile([128, S], BF, tag="sq", name="sq")
                nc.vector.tensor_mul(sq[:], ck(in_bf, k), ck(in_bf, k))
                nc.tensor.matmul(mu_ps[:], onesd[:], ck(in_bf, k),
                                 start=(k == 0), stop=(k == NK - 1))
                nc.tensor.matmul(s2_ps[:], onesd[:], sq[:],
                                 start=(k == 0), stop=(k == NK - 1))
            mu_f = pool_stat.tile([128, S], F32, tag="lnmu", name="mu_f")
            nc.scalar.copy(mu_f[:], mu_ps[:])
            mu2 = pool_rot.tile([128, S], BF, tag="sq", name="mu2")
            nc.vector.tensor_mul(mu2[:], mu_f[:], mu_f[:])
            var = f32tmp()
            nc.vector.tensor_sub(var[:], s2_ps[:], mu2[:])
            sd = f32tmp()
            nc.scalar.activation(sd[:], var[:], AF.Sqrt, bias=epsb[:])
            r_f = pool_stat.tile([128, S], F32, tag="lnr", name="r_f")
            nc.vector.reciprocal_approx_fast(r_f[:], sd[:])
            for k in range(NK):
                cen = pool_rot.tile([128, S], F32, tag="cen", name="cen")
                nc.vector.tensor_sub(cen[:], ck(in_bf, k), mu_f[:])
                nc.vector.tensor_mul(ck(n_bf, k), cen[:], r_f[:])
            return r_f

        # ---- LayerNorm: one tangent's JVP ----
        def ln_tangent(in_bf, tan_bf, n_bf, r_bf, dn_bf):  # r_bf is f32 now
            mt_ps = ps_st.tile([128, S], F32, tag="st", name="mt_ps")
            c_ps = ps_st.tile([128, S], F32, tag="st", name="c_ps")
            for k in range(NK):
                p = pool_rot.tile([128, S], BF, tag="p", name="p")
                nc.vector.tensor_mul(p[:], ck(n_bf, k), ck(tan_bf, k))
                nc.tensor.matmul(mt_ps[:], onesd[:], ck(tan_bf, k),
                                 start=(k == 0), stop=(k == NK - 1))
                nc.tensor.matmul(c_ps[:], onesd[:], p[:],
                                 start=(k == 0), stop=(k == NK - 1))
            ctr = pool_rot.tile([128, S], BF, tag="ctr", name="ctr")
            nc.vector.tensor_mul(ctr[:], c_ps[:], r_bf[:])
            mt_f = pool_rot.tile([128, S], F32, tag="f32tmp", bufs=4, name="mt_f")
            nc.scalar.copy(mt_f[:], mt_ps[:])
            for k in range(NK):
                b_ = pool_rot.tile([128, S], F32, tag="cen", name="b_")
                nc.vector.tensor_sub(b_[:], ck(tan_bf, k), mt_f[:])
                e_ = pool_rot.tile([128, S], F32, tag="e", name="e_")
                nc.vector.tensor_mul(e_[:], b_[:], r_bf[:])
                f_ = pool_rot.tile([128, S], BF, tag="f", name="f_")
                nc.vector.tensor_mul(f_[:], ck(n_bf, k), ctr[:])
                nc.vector.tensor_sub(ck(dn_bf, k), e_[:], f_[:])

        # LN input/output pools outlive the o_sb pool (LIFO nesting); the
        # LN2 tensors reuse the LN1 tags (slot reuse after LN1 consumers end).
        st_n2 = ExitStack()
        pool_lnout = st_n2.enter_context(tc.tile_pool(name="lnout", bufs=1))
        st_ab = ExitStack()
        pool_lnio = st_ab.enter_context(tc.tile_pool(name="lnio", bufs=1))

        st_o = ExitStack()
        pool_o = st_o.enter_context(tc.tile_pool(name="osb", bufs=1))
        o_sb = pool_o.tile([128, NK * S], BF, tag="o", name="o_sb")
        do_sb = [pool_o.tile([128, NK * S], BF, tag=f"do{m}", name=f"do_sb{m}")
                 for m in range(2)]

        st_wp = ExitStack()
        pool_wp = st_wp.enter_context(tc.tile_pool(name="wp", bufs=1))
        wproj_t = []
        for k in range(NK // 2):
            wt = pool_wp.tile([128, D], BF, tag=f"wp{k}", name=f"wp{k}")
            nc.sync.dma_start(wt[:], dram["wproj"][k])
            wproj_t.append(wt)

        st_qkv = ExitStack()
        pool_qkv = st_qkv.enter_context(tc.tile_pool(name="qkv", bufs=1))
        q_sb = pool_qkv.tile([128, NK * S], BF, tag="q", name="q_sb")
        k_sb = pool_qkv.tile([128, NK * S], BF, tag="k", name="k_sb")
        dq_sb = [pool_qkv.tile([128, NK * S], BF, tag=f"dq{m}", name=f"dq_sb{m}")
                 for m in range(2)]
        dk_sb = [pool_qkv.tile([128, NK * S], BF, tag=f"dk{m}", name=f"dk_sb{m}")
                 for m in range(2)]
        v_sb = pool_qkv.tile([128, NSC * D], BF, tag="v", name="v_sb")
        dv_sb = [pool_qkv.tile([128, NSC * D], BF, tag=f"dv{m}", name=f"dv_sb{m}")
                 for m in range(2)]

        # =========== Phases A+B: LN1 and QKV, primal first ===========
        with tc.tile_pool(name="wq", bufs=1) as pool_wq:
            n1 = pool_lnout.tile([128, NK * S], BF, tag="n1", name="n1")
            dn1 = [pool_lnout.tile([128, NK * S], BF, tag=f"dn1{m}",
                                   name=f"dn1{m}") for m in range(2)]
            xbf = pool_lnio.tile([128, NK * S], BF, tag="xbf", name="xbf")
            tbf = [pool_lnio.tile([128, NK * S], BF, tag=f"tbf{m}",
                                  name=f"tbf{m}") for m in range(2)]
            for k in range(NK):
                nc.scalar.dma_start(ck(xbf, k), dram["x_bf"][k])
            wqkv_t = []
            for k in range(NK):
                wt = pool_wq.tile([128, 3 * D], BF, tag=f"wqkv{k}",
                                  name=f"wqkv{k}")
                nc.scalar.dma_start(wt[:], dram["wqkv"][k])
                wqkv_t.append(wt)
            for k in range(NK):
                for m in range(2):
                    nc.sync.dma_start(ck(tbf[m], k), dram["t_bf"][m, k])

            r1_bf = ln_primal(xbf, n1)

            def qk_pass2(src, qd, kd, with_bias):
                for mt in range(12):
                    ps = ps_big.tile([128, S], F32, tag="big", name="qkv_ps")
                    for k in range(NK):
                        nc.tensor.matmul(ps[:],
                                         wqkv_t[k][:, mt * 128:(mt + 1) * 128],
                                         ck(src, k), start=(k == 0),
                                         stop=(k == NK - 1 and not with_bias))
                    if with_bias:
                        nc.tensor.matmul(ps[:],
                                         bqk_sb[0:1, mt * 128:(mt + 1) * 128],
                                         onesrow[:], start=False, stop=True)
                    if mt < 6:
                        nc.scalar.mul(qd[:, mt * S:(mt + 1) * S], ps[:], SCALE)
                    else:
                        nc.scalar.copy(kd[:, (mt - 6) * S:(mt - 5) * S], ps[:])

            def v_pass(src, vd, with_bias):
                for sc in range(NSC):
                    for g2 in range(2):
                        wv_col = 2 * D + g2 * 384
                        ps = ps_big.tile([128, 384], F32, tag="big", name="v_ps")
                        for k in range(NK):
                            lhs = src[:, k * S + sc * 128: k * S + (sc + 1) * 128]
                            nc.tensor.matmul(ps[:], lhs,
                                             wqkv_t[k][:, wv_col:wv_col + 384],
                                             start=(k == 0),
                                             stop=(k == NK - 1 and not with_bias))
                        if with_bias:
                            nc.tensor.matmul(ps[:], ones1[0:1, :],
                                             bv_sb[0:1, g2 * 384:(g2 + 1) * 384],
                                             start=False, stop=True)
                        col = sc * D + g2 * 384
                        nc.scalar.copy(vd[:, col:col + 384], ps[:])

            qk_pass2(n1, q_sb, k_sb, with_bias)
            v_pass(n1, v_sb, with_bias)
            for m in range(2):
                ln_tangent(xbf, tbf[m], n1, r1_bf, dn1[m])
            for m in range(2):
                qk_pass2(dn1[m], dq_sb[m], dk_sb[m], False)
                v_pass(dn1[m], dv_sb[m], False)

        # =========== Phase C: attention, head pairs ===========
        # Heads are processed in pairs (2i, 2i+1) sharing one feature chunk:
        # the K=64 score matmuls of the two heads run concurrently in the PE
        # via row tile_position (0,0)/(64,0); the M=64 attention-value matmuls
        # share one PSUM bank via column tile_position (0,0)/(0,64).
        # Two-stage software pipeline as before, one pair per iteration.
        with tc.tile_pool(name="head2", bufs=2) as pool_h2, \
             tc.tile_pool(name="head1", bufs=1) as pool_h1:
            def stage1(pi):
                kc = pi
                qp = q_sb[:, kc * S:(kc + 1) * S]
                kp = k_sb[:, kc * S:(kc + 1) * S]
                sT = {}
                expT = [pool_h2.tile([128, NSC * S], BF, tag=f"exp{j}",
                                     name=f"expT{j}") for j in range(2)]
                for c in range(NSC):
                    for j, po in ((0, 0), (1, 64)):
                        st = ps_big.tile([128, S], F32, tag="big", name="sT")
                        nc.tensor.matmul(st[:],
                                         kp[po:po + 64, c * 128:(c + 1) * 128],
                                         qp[po:po + 64, :], start=True,
                                         stop=True, tile_position=(po, 0))
                        sT[(j, c)] = st
                    for j in range(2):
                        nc.scalar.activation(ck(expT[j], c), sT[(j, c)][:],
                                             AF.Exp)
                ds_sb = []
                for m in range(2):
                    dqp = dq_sb[m][:, kc * S:(kc + 1) * S]
                    dkp = dk_sb[m][:, kc * S:(kc + 1) * S]
                    dsb = [pool_h2.tile([128, NSC * S], BF, tag=f"ds{m}{j}",
                                        name=f"ds_sb{m}{j}", bufs=1)
                           for j in range(2)]
                    for c in range(NSC):
                        dsp = {}
                        for j, po in ((0, 0), (1, 64)):
                            ps = ps_big.tile([128, S], F32, tag="big",
                                             name="dsT")
                            nc.tensor.matmul(
                                ps[:], dkp[po:po + 64, c * 128:(c + 1) * 128],
                                qp[po:po + 64, :], start=True, stop=False,
                                tile_position=(po, 0))
                            nc.tensor.matmul(
                                ps[:], kp[po:po + 64, c * 128:(c + 1) * 128],
                                dqp[po:po + 64, :], start=False, stop=True,
                                tile_position=(po, 0))
                            dsp[j] = ps
                        for j in range(2):
                            if m == 0:
                                nc.scalar.copy(ck(dsb[j], c), dsp[j][:])
                            else:
                                nc.vector.tensor_copy(ck(dsb[j], c), dsp[j][:])
                    ds_sb.append(dsb)
                return expT, ds_sb

            def stage2(pi, expT, ds_sb):
                kc = pi
                sums = [ps_st.tile([128, S], F32, tag="st", name="sums")
                        for _ in range(2)]
                for j in range(2):
                    for c in range(NSC):
                        nc.tensor.matmul(sums[j][:], ones1[:], ck(expT[j], c),
                                         start=(c == 0), stop=(c == NSC - 1))
                rinv = []
                for j in range(2):
                    rf = f32tmp()
                    nc.vector.reciprocal_approx_fast(rf[:], sums[j][:])
                    rv = pool_h1.tile([128, S], BF, tag=f"rinv{j}",
                                      name=f"rinv{j}")
                    nc.scalar.copy(rv[:], rf[:])
                    rinv.append(rv)
                at = [pool_h1.tile([128, NSC * S], BF, tag=f"at{j}",
                                   name=f"at{j}") for j in range(2)]
                for j in range(2):
                    for c in range(NSC):
                        nc.vector.tensor_mul(ck(at[j], c), ck(expT[j], c),
                                             rinv[j][:])
                o_ps = ps_st.tile([128, S], F32, tag="st", name="o_ps")
                for c in range(NSC):
                    for j, po in ((0, 0), (1, 64)):
                        h = 2 * pi + j
                        vh = v_sb[:, c * D + h * HD: c * D + (h + 1) * HD]
                        nc.tensor.matmul(o_ps[po:po + 64, :], vh, ck(at[j], c),
                                         start=(c == 0), stop=(c == NSC - 1),
                                         tile_position=(0, po))
                osl = o_sb[:, kc * S:(kc + 1) * S]
                nc.scalar.copy(osl, o_ps[:])
                for m in range(2):
                    pt = [pool_h1.tile([128, NSC * S], BF, tag=f"pt{j}",
                                       name=f"pt{j}") for j in range(2)]
                    for j in range(2):
                        for c in range(NSC):
                            nc.vector.tensor_mul(ck(pt[j], c), ck(at[j], c),
                                                 ck(ds_sb[m][j], c))
                    c_ps = [ps_st.tile([128, S], F32, tag="st", name="c_ps2")
                            for _ in range(2)]
                    for j in range(2):
                        for c in range(NSC):
                            nc.tensor.matmul(c_ps[j][:], ones1[:],
                                             ck(pt[j], c), start=(c == 0),
                                             stop=(c == NSC - 1))
                    do_ps = ps_st.tile([128, S], F32, tag="st", name="do_ps")
                    for c in range(NSC):
                        for j, po in ((0, 0), (1, 64)):
                            h = 2 * pi + j
                            vh = v_sb[:, c * D + h * HD: c * D + (h + 1) * HD]
                            dvh = dv_sb[m][:, c * D + h * HD:
                                           c * D + (h + 1) * HD]
                            nc.tensor.matmul(do_ps[po:po + 64, :], vh,
                                             ck(pt[j], c), start=(c == 0),
                                             stop=False, tile_position=(0, po))
                            nc.tensor.matmul(do_ps[po:po + 64, :], dvh,
                                             ck(at[j], c), start=False,
                                             stop=(c == NSC - 1),
                                             tile_position=(0, po))
                    corr = pool_rot.tile([128, S], F32, tag="corr", name="corr", bufs=1)
                    for j, po in ((0, 0), (1, 64)):
                        nc.vector.tensor_mul(corr[po:po + 64, :],
                                             o_sb[po:po + 64,
                                                  kc * S:(kc + 1) * S],
                                             c_ps[j][po:po + 64, :])
                    nc.vector.tensor_sub(do_sb[m][:, kc * S:(kc + 1) * S],
                                         do_ps[:], corr[:])

            pend = None
            for pi in range(H // 2):
                args = stage1(pi)
                if pend is not None:
                    stage2(*pend)
                pend = (pi, *args)
            stage2(*pend)
        st_qkv.close()

        # =========== Phase D: proj + residual (spill xa f32 to DRAM) ========
        with tc.tile_pool(name="resid", bufs=1) as pool_res, \
             tc.tile_pool(name="wp2", bufs=1) as pool_wp2:
            # warm the sqrt table set while the PE is still busy, so LN2's
            # critical path doesn't pay the ACT table load
            warm = pool_rot.tile([128, 1], F32, tag="warm", name="warm")
            nc.scalar.activation(warm[:], epsb[:], AF.Sqrt, bias=epsb[:])
            for k in range(NK // 2, NK):
                wt = pool_wp2.tile([128, D], BF, tag=f"wp{k}", name=f"wp{k}")
                nc.sync.dma_start(wt[:], dram["wproj"][k])
                wproj_t.append(wt)
            # prefetch all residual inputs
            res_in = pool_res.tile([128, 3 * NK * S], F32, tag="resin",
                                   name="res_in")
            for si in range(3):
                for k in range(NK):
                    dst = res_in[:, (si * NK + k) * S:(si * NK + k + 1) * S]
                    if si == 0:
                        nc.scalar.dma_start(dst, dram["x_f32"][k])
                    else:
                        nc.sync.dma_start(dst, dram["t_f32"][si - 1, k])
            xabf = pool_lnio.tile([128, NK * S], BF, tag="xbf", name="xabf")
            dxabf = [pool_lnio.tile([128, NK * S], BF, tag=f"tbf{m}",
                                    name=f"dxabf{m}") for m in range(2)]
            psrcs = [o_sb, do_sb[0], do_sb[1]]
            pdsts = [xabf, dxabf[0], dxabf[1]]
            for mt in range(NK):
                for si in range(3):
                    ps = ps_big.tile([128, S], F32, tag="big", name="pj_ps")
                    for k in range(NK):
                        nc.tensor.matmul(ps[:],
                                         wproj_t[k][:, mt * 128:(mt + 1) * 128],
                                         ck(psrcs[si], k), start=(k == 0),
                                         stop=(k == NK - 1 and
                                               (si != 0 or not with_bias)))
                    if si == 0 and with_bias:
                        nc.tensor.matmul(
                            ps[:], bproj_sb[0:1, mt * 128:(mt + 1) * 128],
                            onesrow[:], start=False, stop=True)
                    xaf = f32tmp()
                    nc.vector.tensor_add(
                        xaf[:], res_in[:, (si * NK + mt) * S:
                                       (si * NK + mt + 1) * S], ps[:])
                    nc.sync.dma_start(xa_scr[si, mt], xaf[:])
                    nc.scalar.copy(ck(pdsts[si], mt), xaf[:])
        st_wp.close()
        st_o.close()

        # =========== Phase E: LN2 (primal first) ===========
        n2 = pool_lnout.tile([128, NK * S], BF, tag="n1", name="n2")
        dn2 = [pool_lnout.tile([128, NK * S], BF, tag=f"dn1{m}", name=f"dn2{m}")
               for m in range(2)]
        r2_bf = ln_primal(xabf, n2)
        ln_tangent(xabf, dxabf[0], n2, r2_bf, dn2[0])

        # =========== Phase F1: W1 + gelu, primal first ===========
        st_g = ExitStack()
        pool_g = st_g.enter_context(tc.tile_pool(name="gq", bufs=1))
        g_sb = pool_g.tile([128, NM1 * S], BF, tag="g", name="g_sb")
        q2_sb = [pool_g.tile([128, NM1 * S], BF, tag=f"q2{m}", name=f"q2_sb{m}")
                 for m in range(2)]
        with tc.tile_pool(name="dgp", bufs=1) as pool_dg:
            for half in range(2):
                dg_sb = pool_dg.tile([128, (NM1 // 2) * S], BF, tag="dg",
                                     name="dg_sb")
                with tc.tile_pool(name=f"w1h{half}", bufs=1) as pool_w1:
                    w1_t = []
                    for k in range(NK):
                        wt = pool_w1.tile([128, DFF // 2], BF, tag=f"w1{k}",
                                          name=f"w1_{half}_{k}")
                        nc.sync.dma_start(
                            wt[:], dram["w1"][k][:, half * (DFF // 2):
                                                 (half + 1) * (DFF // 2)])
                        w1_t.append(wt)
                    for mi in range(NM1 // 2):
                        mt = half * (NM1 // 2) + mi
                        ps = ps_big.tile([128, S], F32, tag="big", name="u_ps")
                        for k in range(NK):
                            nc.tensor.matmul(ps[:],
                                             w1_t[k][:, mi * 128:(mi + 1) * 128],
                                             ck(n2, k), start=(k == 0),
                                             stop=(k == NK - 1 and
                                                   not with_bias))
                        if with_bias:
                            nc.tensor.matmul(
                                ps[:], b1m_sb[0:1, mt * 128:(mt + 1) * 128],
                                onesrow[:], start=False, stop=True)
                        nc.scalar.activation(ck(g_sb, mt), ps[:], AF.Gelu)
                        nc.scalar.activation(ck(dg_sb, mi), ps[:],
                                             AF.Derivative_Gelu)
                    if half == 0:
                        ln_tangent(xabf, dxabf[1], n2, r2_bf, dn2[1])
                    for m in range(2):
                        for mi in range(NM1 // 2):
                            mt = half * (NM1 // 2) + mi
                            ps = ps_big.tile([128, S], F32, tag="big",
                                             name="ut_ps")
                            for k in range(NK):
                                nc.tensor.matmul(
                                    ps[:], w1_t[k][:, mi * 128:(mi + 1) * 128],
                                    ck(dn2[m], k), start=(k == 0),
                                    stop=(k == NK - 1))
                            nc.vector.tensor_mul(ck(q2_sb[m], mt),
                                                 ck(dg_sb, mi), ps[:])

        # =========== Phase F2: W2 + final residual ===========
        fsrcs = [g_sb, q2_sb[0], q2_sb[1]]
        with tc.tile_pool(name="w2p", bufs=1) as pool_w2, \
             tc.tile_pool(name="res2", bufs=1) as pool_res2:
            w2_t = []
            for k in range(NM1):
                wt = pool_w2.tile([128, D], BF, tag=f"w2{k}", name=f"w2_{k}")
                nc.scalar.dma_start(wt[:], dram["w2"][k])
                w2_t.append(wt)
            for si in range(3):
                for mt in range(NK):
                    res2 = pool_res2.tile([128, S], F32, tag="res2", bufs=6,
                                          name="res2")
                    nc.sync.dma_start(res2[:], xa_scr[si, mt])
                    ps = ps_big.tile([128, S], F32, tag="big", name="o2_ps")
                    for k in range(NM1):
                        nc.tensor.matmul(ps[:],
                                         w2_t[k][:, mt * 128:(mt + 1) * 128],
                                         ck(fsrcs[si], k), start=(k == 0),
                                         stop=(k == NM1 - 1))
                    ot = f32tmp()
                    nc.vector.tensor_add(ot[:], res2[:], ps[:])
                    nc.sync.dma_start(out_d[si, mt], ot[:])
        st_g.close()
        st_ab.close()
        st_n2.close()


def _prep_host(inputs):
    f32 = np.float32
    x = np.asarray(inputs["x"], f32)
    xt = np.asarray(inputs["x_tangent"], f32)
    g1 = np.asarray(inputs["g1"], f32); b1 = np.asarray(inputs["b1"], f32)
    g2 = np.asarray(inputs["g2"], f32); b2 = np.asarray(inputs["b2"], f32)
    Wqkv = np.asarray(inputs["Wqkv"], f32); Wproj = np.asarray(inputs["Wproj"], f32)
    W1 = np.asarray(inputs["W1"], f32); W2 = np.asarray(inputs["W2"], f32)
    bproj = np.asarray(inputs["bproj"], f32)
    bf1 = np.asarray(inputs["bf1"], f32); bf2 = np.asarray(inputs["bf2"], f32)

    bf16 = ml_dtypes.bfloat16
    Wqkv_f = g1[:, None] * Wqkv
    bqkv = b1 @ Wqkv
    W1_f = g2[:, None] * W1
    b1m = b2 @ W1 + bf1

    def tile_k(w, nk):
        return np.ascontiguousarray(w.reshape(nk, 128, -1))

    shared = {
        "wqkv": tile_k(Wqkv_f, NK).astype(bf16),
        "wproj": tile_k(Wproj, NK).astype(bf16),
        "w1": tile_k(W1_f, NK).astype(bf16),
        "w2": tile_k(W2, NM1).astype(bf16),
        "bqk": np.ascontiguousarray(bqkv[None, :2 * D]).astype(bf16),
        "bv": np.ascontiguousarray(bqkv[None, 2 * D:]).astype(bf16),
        "bproj": np.ascontiguousarray(bproj[None, :]).astype(bf16),
        "b1m": np.ascontiguousarray(b1m[None, :]).astype(bf16),
    }
    in_maps = []
    for core in range(N_CORES):
        b, mp = core // 2, core % 2
        xT = np.ascontiguousarray(x[b].T).reshape(NK, 128, S)
        tT = np.ascontiguousarray(
            xt[b, 2 * mp:2 * mp + 2].transpose(0, 2, 1)).reshape(2, NK, 128, S)
        im = dict(shared)
        im["x_f32"] = xT
        im["x_bf"] = xT.astype(bf16)
        im["t_f32"] = tT
        im["t_bf"] = tT.astype(bf16)
        in_maps.append(im)
    return in_maps, bf2


def kernel(**inputs):
    with_bias = not all(
        np.allclose(np.asarray(inputs[k]), 0.0)
        for k in ("b1", "bproj", "b2", "bf1"))
    key = ("nc", with_bias)
    if key not in _CACHE:
        _CACHE[key] = _build_program(with_bias)
    nc = _CACHE[key]
    in_maps, bf2 = _prep_host(inputs)
    res = run_bass_kernel_spmd(nc, in_maps, core_ids=list(range(N_CORES)),
                               **_RUN_KWARGS)
    _LAST_RES[0] = res
    out = np.zeros((B, S, D), np.float32)
    out_tan = np.zeros((B, M, S, D), np.float32)
    for core in range(N_CORES):
        b, mp = core // 2, core % 2
        o = res.results[core]["out"].reshape(3, D, S)
        if mp == 0:
            out[b] = o[0].T + bf2[None, :]
        out_tan[b, 2 * mp] = o[1].T
        out_tan[b, 2 * mp + 1] = o[2].T
    return out, out_tan


# revision 26
# speedup vs baseline: 1.2408x; 1.0226x over previous
# Trainium2 Bass kernel for a pre-norm transformer block with forward-mode JVP
# (jax.linearize) over M=4 tangent directions.
#
# Sharding: 8 cores; core c handles batch b=c//2 and tangents {2*(c%2), 2*(c%2)+1}.
# Each core computes the primal pass for its batch (even/odd core pairs do this
# redundantly; the even core's primal is used) plus 2 tangent JVP passes.
# No cross-core communication.
#
# On-chip layout is feature-major ([D, S] with features on partitions), so every
# linear layer is a plain accumulated matmul with no transposes. LayerNorm /
# softmax statistics (which reduce over partitions in this layout) are computed
# on the PE via ones-matmuls whose M=128 output broadcasts the column sums to
# all partitions. Softmax is computed without max-subtraction (scores are O(3)
# for this problem's data distribution). LN affine (g, b) is folded into the
# weights on the host; biases enter via K=1 ones-row matmuls; the final mlp
# bias bf2 is added on the host.
#
# Matmul inputs are bf16 (fp32 PSUM accumulation); the residual stream and LN /
# softmax statistics stay fp32 (residual adds read the fp32 inputs re-DMAed
# from DRAM; xa is spilled to DRAM scratch between the attention and MLP
# residual adds to stay under the SBUF budget).

import numpy as np
import ml_dtypes

import concourse.bass as bass
import concourse.tile as tile
from concourse import bacc, mybir
from concourse.bass_utils import run_bass_kernel_spmd

AF = mybir.ActivationFunctionType
BF = mybir.dt.bfloat16
F32 = mybir.dt.float32

B, S, D, H, M = 4, 512, 768, 12, 4
DFF = 4 * D
HD = D // H
EPS = 1e-6
SCALE = HD ** -0.5
NK = D // 128          # 6 feature chunks
NSC = S // 128         # 4 sequence chunks
NM1 = DFF // 128       # 24
N_CORES = 8

_CACHE = {}
_RUN_KWARGS = {}   # test harness can set {"trace": True}
_LAST_RES = [None]


def _build_program(with_bias):
    nc = bacc.Bacc("TRN2", target_bir_lowering=False, debug=False,
                   num_devices=N_CORES)

    dram = {}
    def din(name, shape, dt):
        dram[name] = nc.dram_tensor(name, shape, dt, kind="ExternalInput").ap()
    din("x_f32", [NK, 128, S], F32)
    din("x_bf", [NK, 128, S], BF)
    din("t_f32", [2, NK, 128, S], F32)
    din("t_bf", [2, NK, 128, S], BF)
    din("wqkv", [NK, 128, 3 * D], BF)
    din("wproj", [NK, 128, D], BF)
    din("w1", [NK, 128, DFF], BF)
    din("w2", [NM1, 128, D], BF)
    din("bqk", [1, 2 * D], BF)
    din("bv", [1, D], BF)
    din("bproj", [1, D], BF)
    din("b1m", [1, DFF], BF)
    out_d = nc.dram_tensor("out", [3, NK, 128, S], F32, kind="ExternalOutput").ap()
    xa_scr = nc.dram_tensor("xa_scr", [3, NK, 128, S], F32).ap()
    xab_scr = nc.dram_tensor("xab_scr", [3, NK, 128, S], BF).ap()

    with tile.TileContext(nc) as tc:
        _emit(nc, tc, dram, out_d, xa_scr, xab_scr, with_bias)
    nc.compile()
    return nc


def _emit(nc, tc, dram, out_d, xa_scr, xab_scr, with_bias):
    from contextlib import ExitStack
    ctx = ExitStack()
    with ctx:
        pool_const = ctx.enter_context(tc.tile_pool(name="const", bufs=1))
        pool_rot = ctx.enter_context(tc.tile_pool(name="rot", bufs=2))
        pool_stat = ctx.enter_context(tc.tile_pool(name="stat", bufs=1))
        ps_big = ctx.enter_context(tc.tile_pool(name="psbig", bufs=5, space="PSUM"))
        ps_st = ctx.enter_context(tc.tile_pool(name="psst", bufs=3, space="PSUM"))

        # ---- constants ----
        ones1 = pool_const.tile([128, 128], BF, tag="ones1")
        nc.gpsimd.memset(ones1[:], 1.0)
        onesd = pool_const.tile([128, 128], BF, tag="onesd")
        nc.gpsimd.memset(onesd[:], 1.0 / D)
        onesrow = pool_const.tile([1, S], BF, tag="onesrow")
        nc.gpsimd.memset(onesrow[:], 1.0)
        epsb = pool_const.tile([128, 1], F32, tag="epsb")
        nc.gpsimd.memset(epsb[:], EPS)
        bqk_sb = pool_const.tile([1, 2 * D], BF, tag="bqk")
        nc.sync.dma_start(bqk_sb[:], dram["bqk"][:])
        bv_sb = pool_const.tile([1, D], BF, tag="bv")
        nc.sync.dma_start(bv_sb[:], dram["bv"][:])
        bproj_sb = pool_const.tile([1, D], BF, tag="bproj")
        nc.sync.dma_start(bproj_sb[:], dram["bproj"][:])
        b1m_sb = pool_const.tile([1, DFF], BF, tag="b1m")
        nc.sync.dma_start(b1m_sb[:], dram["b1m"][:])

        warm0 = pool_const.tile([128, 1], F32, tag="warm0")
        nc.scalar.activation(warm0[:], epsb[:], AF.Sqrt, bias=epsb[:])

        def ck(t, k):
            return t[:, k * S:(k + 1) * S]

        def f32tmp():
            return pool_rot.tile([128, S], F32, tag="f32tmp", bufs=4,
                                 name="f32tmp")

        # ---- LayerNorm: primal part (stats via PE colsum-broadcast) ----
        def ln_primal(in_bf, n_bf):
            mu_ps = ps_st.tile([128, S], F32, tag="st", name="mu_ps")
            s2_ps = ps_st.tile([128, S], F32, tag="st", name="s2_ps")
            for k in range(NK):
                sq = pool_rot.tile([128, S], BF, tag="sq", name="sq")
                nc.vector.tensor_mul(sq[:], ck(in_bf, k), ck(in_bf, k))
                nc.tensor.matmul(mu_ps[:], onesd[:], ck(in_bf, k),
                                 start=(k == 0), stop=(k == NK - 1))
                nc.tensor.matmul(s2_ps[:], onesd[:], sq[:],
                                 start=(k == 0), stop=(k == NK - 1))
            mu_f = pool_stat.tile([128, S], F32, tag="lnmu", name="mu_f")
            nc.scalar.copy(mu_f[:], mu_ps[:])
            mu2 = pool_rot.tile([128, S], BF, tag="sq", name="mu2")
            nc.vector.tensor_mul(mu2[:], mu_f[:], mu_f[:])
            var = f32tmp()
            nc.vector.tensor_sub(var[:], s2_ps[:], mu2[:])
            sd = f32tmp()
            nc.scalar.activation(sd[:], var[:], AF.Sqrt, bias=epsb[:])
            r_f = pool_stat.tile([128, S], F32, tag="lnr", name="r_f")
            nc.vector.reciprocal_approx_fast(r_f[:], sd[:])
            for k in range(NK):
                cen = pool_rot.tile([128, S], F32, tag="cen", name="cen")
                nc.vector.tensor_sub(cen[:], ck(in_bf, k), mu_f[:])
                nc.vector.tensor_mul(ck(n_bf, k), cen[:], r_f[:])
            return r_f

        # ---- LayerNorm: one tangent's JVP ----
        def ln_tangent(in_bf, tan_bf, n_bf, r_bf, dn_bf):  # r_bf is f32 now
            mt_ps = ps_st.tile([128, S], F32, tag="st", name="mt_ps")
            c_ps = ps_st.tile([128, S], F32, tag="st", name="c_ps")
            for k in range(NK):
                p = pool_rot.tile([128, S], BF, tag="p", name="p")
                nc.vector.tensor_mul(p[:], ck(n_bf, k), ck(tan_bf, k))
                nc.tensor.matmul(mt_ps[:], onesd[:], ck(tan_bf, k),
                                 start=(k == 0), stop=(k == NK - 1))
                nc.tensor.matmul(c_ps[:], onesd[:], p[:],
                                 start=(k == 0), stop=(k == NK - 1))
            ctr = pool_rot.tile([128, S], BF, tag="ctr", name="ctr")
            nc.vector.tensor_mul(ctr[:], c_ps[:], r_bf[:])
            mt_f = pool_rot.tile([128, S], F32, tag="f32tmp", bufs=4, name="mt_f")
            nc.scalar.copy(mt_f[:], mt_ps[:])
            for k in range(NK):
                b_ = pool_rot.tile([128, S], F32, tag="cen", name="b_")
                nc.vector.tensor_sub(b_[:], ck(tan_bf, k), mt_f[:])
                e_ = pool_rot.tile([128, S], F32, tag="e", name="e_")
                nc.vector.tensor_mul(e_[:], b_[:], r_bf[:])
                f_ = pool_rot.tile([128, S], BF, tag="f", name="f_")
                nc.vector.tensor_mul(f_[:], ck(n_bf, k), ctr[:])
                nc.vector.tensor_sub(ck(dn_bf, k), e_[:], f_[:])

        # LN input/output pools outlive the o_sb pool (LIFO nesting); the
        # LN2 tensors reuse the LN1 tags (slot reuse after LN1 consumers end).
        st_n2 = ExitStack()
        pool_lnout = st_n2.enter_context(tc.tile_pool(name="lnout", bufs=1))
        st_ab = ExitStack()
        pool_lnio = st_ab.enter_context(tc.tile_pool(name="lnio", bufs=1))

        st_o = ExitStack()
        pool_o = st_o.enter_context(tc.tile_pool(name="osb", bufs=1))
        o_sb = pool_o.tile([128, NK * S], BF, tag="o", name="o_sb")
        do_sb = [pool_o.tile([128, NK * S], BF, tag=f"do{m}", name=f"do_sb{m}")
                 for m in range(2)]

        st_wp = ExitStack()
        pool_wp = st_wp.enter_context(tc.tile_pool(name="wp", bufs=1))
        wproj_t = []
        for k in range(NK // 2):
            wt = pool_wp.tile([128, D], BF, tag=f"wp{k}", name=f"wp{k}")
            nc.sync.dma_start(wt[:], dram["wproj"][k])
            wproj_t.append(wt)

        st_qkv = ExitStack()
        pool_qkv = st_qkv.enter_context(tc.tile_pool(name="qkv", bufs=1))
        q_sb = pool_qkv.tile([128, NK * S], BF, tag="q", name="q_sb")
        k_sb = pool_qkv.tile([128, NK * S], BF, tag="k", name="k_sb")
        dq_sb = [pool_qkv.tile([128, NK * S], BF, tag=f"dq{m}", name=f"dq_sb{m}")
                 for m in range(2)]
        dk_sb = [pool_qkv.tile([128, NK * S], BF, tag=f"dk{m}", name=f"dk_sb{m}")
                 for m in range(2)]
        v_sb = pool_qkv.tile([128, NSC * D], BF, tag="v", name="v_sb")
        dv_sb = [pool_qkv.tile([128, NSC * D], BF, tag=f"dv{m}", name=f"dv_sb{m}")
                 for m in range(2)]

        # =========== Phases A+B: LN1 and QKV, primal first ===========
        with tc.tile_pool(name="wq", bufs=1) as pool_wq:
            n1 = pool_lnout.tile([128, NK * S], BF, tag="n1", name="n1")
            dn1 = [pool_lnout.tile([128, NK * S], BF, tag=f"dn1{m}",
                                   name=f"dn1{m}") for m in range(2)]
            xbf = pool_lnio.tile([128, NK * S], BF, tag="xbf", name="xbf")
            tbf = [pool_lnio.tile([128, NK * S], BF, tag=f"tbf{m}",
                                  name=f"tbf{m}") for m in range(2)]
            for k in range(NK):
                nc.scalar.dma_start(ck(xbf, k), dram["x_bf"][k])
            wqkv_t = []
            for k in range(NK):
                wt = pool_wq.tile([128, 3 * D], BF, tag=f"wqkv{k}",
                                  name=f"wqkv{k}")
                nc.scalar.dma_start(wt[:], dram["wqkv"][k])
                wqkv_t.append(wt)
            for k in range(NK):
                for m in range(2):
                    nc.sync.dma_start(ck(tbf[m], k), dram["t_bf"][m, k])

            r1_bf = ln_primal(xbf, n1)

            def qk_pass2(src, qd, kd, with_bias):
                for mt in range(12):
                    ps = ps_big.tile([128, S], F32, tag="big", name="qkv_ps")
                    for k in range(NK):
                        nc.tensor.matmul(ps[:],
                                         wqkv_t[k][:, mt * 128:(mt + 1) * 128],
                                         ck(src, k), start=(k == 0),
                                         stop=(k == NK - 1 and not with_bias))
                    if with_bias:
                        nc.tensor.matmul(ps[:],
                                         bqk_sb[0:1, mt * 128:(mt + 1) * 128],
                                         onesrow[:], start=False, stop=True)
                    if mt < 6:
                        nc.scalar.mul(qd[:, mt * S:(mt + 1) * S], ps[:], SCALE)
                    else:
                        nc.scalar.copy(kd[:, (mt - 6) * S:(mt - 5) * S], ps[:])

            def v_pass(src, vd, with_bias):
                for sc in range(NSC):
                    for g2 in range(2):
                        wv_col = 2 * D + g2 * 384
                        ps = ps_big.tile([128, 384], F32, tag="big", name="v_ps")
                        for k in range(NK):
                            lhs = src[:, k * S + sc * 128: k * S + (sc + 1) * 128]
                            nc.tensor.matmul(ps[:], lhs,
                                             wqkv_t[k][:, wv_col:wv_col + 384],
                                             start=(k == 0),
                                             stop=(k == NK - 1 and not with_bias))
                        if with_bias:
                            nc.tensor.matmul(ps[:], ones1[0:1, :],
                                             bv_sb[0:1, g2 * 384:(g2 + 1) * 384],
                                             start=False, stop=True)
                        col = sc * D + g2 * 384
                        nc.scalar.copy(vd[:, col:col + 384], ps[:])

            qk_pass2(n1, q_sb, k_sb, with_bias)
            v_pass(n1, v_sb, with_bias)
            for m in range(2):
                ln_tangent(xbf, tbf[m], n1, r1_bf, dn1[m])
            for m in range(2):
                qk_pass2(dn1[m], dq_sb[m], dk_sb[m], False)
                v_pass(dn1[m], dv_sb[m], False)

        # =========== Phase C: attention, head pairs ===========
        # Heads are processed in pairs (2i, 2i+1) sharing one feature chunk:
        # the K=64 score matmuls of the two heads run concurrently in the PE
        # via row tile_position (0,0)/(64,0); the M=64 attention-value matmuls
        # share one PSUM bank via column tile_position (0,0)/(0,64).
        # Two-stage software pipeline as before, one pair per iteration.
        with tc.tile_pool(name="head2", bufs=2) as pool_h2, \
             tc.tile_pool(name="head1", bufs=1) as pool_h1:
            def stage1(pi):
                kc = pi
                qp = q_sb[:, kc * S:(kc + 1) * S]
                kp = k_sb[:, kc * S:(kc + 1) * S]
                sT = {}
                expT = [pool_h2.tile([128, NSC * S], BF, tag=f"exp{j}",
                                     name=f"expT{j}") for j in range(2)]
                for c in range(NSC):
                    for j, po in ((0, 0), (1, 64)):
                        st = ps_big.tile([128, S], F32, tag="big", name="sT")
                        nc.tensor.matmul(st[:],
                                         kp[po:po + 64, c * 128:(c + 1) * 128],
                                         qp[po:po + 64, :], start=True,
                                         stop=True, tile_position=(po, 0))
                        sT[(j, c)] = st
                    for j in range(2):
                        nc.scalar.activation(ck(expT[j], c), sT[(j, c)][:],
                                             AF.Exp)
                ds_sb = []
                for m in range(2):
                    dqp = dq_sb[m][:, kc * S:(kc + 1) * S]
                    dkp = dk_sb[m][:, kc * S:(kc + 1) * S]
                    dsb = [pool_h2.tile([128, NSC * S], BF, tag=f"ds{m}{j}",
                                        name=f"ds_sb{m}{j}", bufs=1)
                           for j in range(2)]
                    for c in range(NSC):
                        dsp = {}
                        for j, po in ((0, 0), (1, 64)):
                            ps = ps_big.tile([128, S], F32, tag="big",
                                             name="dsT")
                            nc.tensor.matmul(
                                ps[:], dkp[po:po + 64, c * 128:(c + 1) * 128],
                                qp[po:po + 64, :], start=True, stop=False,
                                tile_position=(po, 0))
                            nc.tensor.matmul(
                                ps[:], kp[po:po + 64, c * 128:(c + 1) * 128],
                                dqp[po:po + 64, :], start=False, stop=True,
                                tile_position=(po, 0))
                            dsp[j] = ps
                        for j in range(2):
                            if m == 0 or (c + j) % 2 == 0:
                                nc.scalar.copy(ck(dsb[j], c), dsp[j][:])
                            else:
                                nc.vector.tensor_copy(ck(dsb[j], c), dsp[j][:])
                    ds_sb.append(dsb)
                return expT, ds_sb

            def stage2(pi, expT, ds_sb):
                kc = pi
                sums = [ps_st.tile([128, S], F32, tag="st", name="sums")
                        for _ in range(2)]
                for j in range(2):
                    for c in range(NSC):
                        nc.tensor.matmul(sums[j][:], ones1[:], ck(expT[j], c),
                                         start=(c == 0), stop=(c == NSC - 1))
                rinv = []
                for j in range(2):
                    rf = f32tmp()
                    nc.vector.reciprocal_approx_fast(rf[:], sums[j][:])
                    rv = pool_h1.tile([128, S], BF, tag=f"rinv{j}",
                                      name=f"rinv{j}")
                    nc.scalar.copy(rv[:], rf[:])
                    rinv.append(rv)
                at = [pool_h1.tile([128, NSC * S], BF, tag=f"at{j}",
                                   name=f"at{j}") for j in range(2)]
                for j in range(2):
                    for c in range(NSC):
                        nc.vector.tensor_mul(ck(at[j], c), ck(expT[j], c),
                                             rinv[j][:])
                o_ps = ps_st.tile([128, S], F32, tag="st", name="o_ps")
                for c in range(NSC):
                    for j, po in ((0, 0), (1, 64)):
                        h = 2 * pi + j
                        vh = v_sb[:, c * D + h * HD: c * D + (h + 1) * HD]
                        nc.tensor.matmul(o_ps[po:po + 64, :], vh, ck(at[j], c),
                                         start=(c == 0), stop=(c == NSC - 1),
                                         tile_position=(0, po))
                osl = o_sb[:, kc * S:(kc + 1) * S]
                nc.scalar.copy(osl, o_ps[:])
                for m in range(2):
                    pt = [pool_h1.tile([128, NSC * S], BF, tag=f"pt{j}",
                                       name=f"pt{j}") for j in range(2)]
                    for j in range(2):
                        for c in range(NSC):
                            nc.vector.tensor_mul(ck(pt[j], c), ck(at[j], c),
                                                 ck(ds_sb[m][j], c))
                    c_ps = [ps_st.tile([128, S], F32, tag="st", name="c_ps2")
                            for _ in range(2)]
                    for j in range(2):
                        for c in range(NSC):
                            nc.tensor.matmul(c_ps[j][:], ones1[:],
                                             ck(pt[j], c), start=(c == 0),
                                             stop=(c == NSC - 1))
                    do_ps = ps_st.tile([128, S], F32, tag="st", name="do_ps")
                    for c in range(NSC):
                        for j, po in ((0, 0), (1, 64)):
                            h = 2 * pi + j
                            vh = v_sb[:, c * D + h * HD: c * D + (h + 1) * HD]
                            dvh = dv_sb[m][:, c * D + h * HD:
                                           c * D + (h + 1) * HD]
                            nc.tensor.matmul(do_ps[po:po + 64, :], vh,
                                             ck(pt[j], c), start=(c == 0),
                                             stop=False, tile_position=(0, po))
                            nc.tensor.matmul(do_ps[po:po + 64, :], dvh,
                                             ck(at[j], c), start=False,
                                             stop=(c == NSC - 1),
                                             tile_position=(0, po))
                    corr = pool_rot.tile([128, S], F32, tag="corr", name="corr", bufs=1)
                    for j, po in ((0, 0), (1, 64)):
                        nc.vector.tensor_mul(corr[po:po + 64, :],
                                             o_sb[po:po + 64,
                                                  kc * S:(kc + 1) * S],
                                             c_ps[j][po:po + 64, :])
                    nc.vector.tensor_sub(do_sb[m][:, kc * S:(kc + 1) * S],
                                         do_ps[:], corr[:])

            pend = None
            for pi in range(H // 2):
                args = stage1(pi)
                if pend is not None:
                    stage2(*pend)
                pend = (pi, *args)
            stage2(*pend)
        st_qkv.close()

        # =========== Phase D: proj + residual (spill xa f32 to DRAM) ========
        with tc.tile_pool(name="resid", bufs=1) as pool_res, \
             tc.tile_pool(name="wp2", bufs=1) as pool_wp2:
            # warm the sqrt table set while the PE is still busy, so LN2's
            # critical path doesn't pay the ACT table load
            warm = pool_rot.tile([128, 1], F32, tag="warm", name="warm")
            nc.scalar.activation(warm[:], epsb[:], AF.Sqrt, bias=epsb[:])
            for k in range(NK // 2, NK):
                wt = pool_wp2.tile([128, D], BF, tag=f"wp{k}", name=f"wp{k}")
                nc.sync.dma_start(wt[:], dram["wproj"][k])
                wproj_t.append(wt)
            # prefetch all residual inputs
            res_in = pool_res.tile([128, 3 * NK * S], F32, tag="resin",
                                   name="res_in")
            for si in range(3):
                for k in range(NK):
                    dst = res_in[:, (si * NK + k) * S:(si * NK + k + 1) * S]
                    if si == 0:
                        nc.scalar.dma_start(dst, dram["x_f32"][k])
                    else:
                        nc.sync.dma_start(dst, dram["t_f32"][si - 1, k])
            xabf = pool_lnio.tile([128, NK * S], BF, tag="xbf", name="xabf")
            dxabf = [pool_lnio.tile([128, NK * S], BF, tag=f"tbf{m}",
                                    name=f"dxabf{m}") for m in range(2)]
            psrcs = [o_sb, do_sb[0], do_sb[1]]
            pdsts = [xabf, dxabf[0], dxabf[1]]
            for mt in range(NK):
                for si in range(3):
                    ps = ps_big.tile([128, S], F32, tag="big", name="pj_ps")
                    for k in range(NK):
                        nc.tensor.matmul(ps[:],
                                         wproj_t[k][:, mt * 128:(mt + 1) * 128],
                                         ck(psrcs[si], k), start=(k == 0),
                                         stop=(k == NK - 1 and
                                               (si != 0 or not with_bias)))
                    if si == 0 and with_bias:
                        nc.tensor.matmul(
                            ps[:], bproj_sb[0:1, mt * 128:(mt + 1) * 128],
                            onesrow[:], start=False, stop=True)
                    xaf = f32tmp()
                    nc.vector.tensor_add(
                        xaf[:], res_in[:, (si * NK + mt) * S:
                                       (si * NK + mt + 1) * S], ps[:])
                    nc.sync.dma_start(xa_scr[si, mt], xaf[:])
                    nc.scalar.copy(ck(pdsts[si], mt), xaf[:])
        st_wp.close()
        st_o.close()

        # =========== Phase E: LN2 (primal first) ===========
        n2 = pool_lnout.tile([128, NK * S], BF, tag="n1", name="n2")
        dn2 = [pool_lnout.tile([128, NK * S], BF, tag=f"dn1{m}", name=f"dn2{m}")
               for m in range(2)]
        r2_bf = ln_primal(xabf, n2)
        ln_tangent(xabf, dxabf[0], n2, r2_bf, dn2[0])

        # =========== Phase F1: W1 + gelu, primal first ===========
        st_g = ExitStack()
        pool_g = st_g.enter_context(tc.tile_pool(name="gq", bufs=1))
        g_sb = pool_g.tile([128, NM1 * S], BF, tag="g", name="g_sb")
        q2_sb = [pool_g.tile([128, NM1 * S], BF, tag=f"q2{m}", name=f"q2_sb{m}")
                 for m in range(2)]
        with tc.tile_pool(name="dgp", bufs=1) as pool_dg:
            for half in range(2):
                dg_sb = pool_dg.tile([128, (NM1 // 2) * S], BF, tag="dg",
                                     name="dg_sb")
                with tc.tile_pool(name=f"w1h{half}", bufs=1) as pool_w1:
                    w1_t = []
                    for k in range(NK):
                        wt = pool_w1.tile([128, DFF // 2], BF, tag=f"w1{k}",
                                          name=f"w1_{half}_{k}")
                        nc.sync.dma_start(
                            wt[:], dram["w1"][k][:, half * (DFF // 2):
                                                 (half + 1) * (DFF // 2)])
                        w1_t.append(wt)
                    for mi in range(NM1 // 2):
                        mt = half * (NM1 // 2) + mi
                        ps = ps_big.tile([128, S], F32, tag="big", name="u_ps")
                        for k in range(NK):
                            nc.tensor.matmul(ps[:],
                                             w1_t[k][:, mi * 128:(mi + 1) * 128],
                                             ck(n2, k), start=(k == 0),
                                             stop=(k == NK - 1 and
                                                   not with_bias))
                        if with_bias:
                            nc.tensor.matmul(
                                ps[:], b1m_sb[0:1, mt * 128:(mt + 1) * 128],
                                onesrow[:], start=False, stop=True)
                        nc.scalar.activation(ck(g_sb, mt), ps[:], AF.Gelu)
                        nc.scalar.activation(ck(dg_sb, mi), ps[:],
                                             AF.Derivative_Gelu)
                    if half == 0:
                        ln_tangent(xabf, dxabf[1], n2, r2_bf, dn2[1])
                    for m in range(2):
                        for mi in range(NM1 // 2):
                            mt = half * (NM1 // 2) + mi
                            ps = ps_big.tile([128, S], F32, tag="big",
                                             name="ut_ps")
                            for k in range(NK):
                                nc.tensor.matmul(
                                    ps[:], w1_t[k][:, mi * 128:(mi + 1) * 128],
                                    ck(dn2[m], k), start=(k == 0),
                                    stop=(k == NK - 1))
                            nc.vector.tensor_mul(ck(q2_sb[m], mt),
                                                 ck(dg_sb, mi), ps[:])

        # =========== Phase F2: W2 + final residual ===========
        fsrcs = [g_sb, q2_sb[0], q2_sb[1]]
        with tc.tile_pool(name="w2p", bufs=1) as pool_w2, \
             tc.tile_pool(name="res2", bufs=1) as pool_res2:
            w2_t = []
            for k in range(NM1):
                wt = pool_w2.tile([128, D], BF, tag=f"w2{k}", name=f"w2_{k}")
                nc.scalar.dma_start(wt[:], dram["w2"][k])
                w2_t.append(wt)
            for si in range(3):
                for mt in range(NK):
                    res2 = pool_res2.tile([128, S], F32, tag="res2", bufs=6,
                                          name="res2")
                    nc.sync.dma_start(res2[:], xa_scr[si, mt])
                    ps = ps_big.tile([128, S], F32, tag="big", name="o2_ps")
                    for k in range(NM1):
                        nc.tensor.matmul(ps[:],
                                         w2_t[k][:, mt * 128:(mt + 1) * 128],
                                         ck(fsrcs[si], k), start=(k == 0),
                                         stop=(k == NM1 - 1))
                    ot = f32tmp()
                    nc.vector.tensor_add(ot[:], res2[:], ps[:])
                    nc.sync.dma_start(out_d[si, mt], ot[:])
        st_g.close()
        st_ab.close()
        st_n2.close()


def _prep_host(inputs):
    f32 = np.float32
    x = np.asarray(inputs["x"], f32)
    xt = np.asarray(inputs["x_tangent"], f32)
    g1 = np.asarray(inputs["g1"], f32); b1 = np.asarray(inputs["b1"], f32)
    g2 = np.asarray(inputs["g2"], f32); b2 = np.asarray(inputs["b2"], f32)
    Wqkv = np.asarray(inputs["Wqkv"], f32); Wproj = np.asarray(inputs["Wproj"], f32)
    W1 = np.asarray(inputs["W1"], f32); W2 = np.asarray(inputs["W2"], f32)
    bproj = np.asarray(inputs["bproj"], f32)
    bf1 = np.asarray(inputs["bf1"], f32); bf2 = np.asarray(inputs["bf2"], f32)

    bf16 = ml_dtypes.bfloat16
    Wqkv_f = g1[:, None] * Wqkv
    bqkv = b1 @ Wqkv
    W1_f = g2[:, None] * W1
    b1m = b2 @ W1 + bf1

    def tile_k(w, nk):
        return np.ascontiguousarray(w.reshape(nk, 128, -1))

    shared = {
        "wqkv": tile_k(Wqkv_f, NK).astype(bf16),
        "wproj": tile_k(Wproj, NK).astype(bf16),
        "w1": tile_k(W1_f, NK).astype(bf16),
        "w2": tile_k(W2, NM1).astype(bf16),
        "bqk": np.ascontiguousarray(bqkv[None, :2 * D]).astype(bf16),
        "bv": np.ascontiguousarray(bqkv[None, 2 * D:]).astype(bf16),
        "bproj": np.ascontiguousarray(bproj[None, :]).astype(bf16),
        "b1m": np.ascontiguousarray(b1m[None, :]).astype(bf16),
    }
    in_maps = []
    for core in range(N_CORES):
        b, mp = core // 2, core % 2
        xT = np.ascontiguousarray(x[b].T).reshape(NK, 128, S)
        tT = np.ascontiguousarray(
            xt[b, 2 * mp:2 * mp + 2].transpose(0, 2, 1)).reshape(2, NK, 128, S)
        im = dict(shared)
        im["x_f32"] = xT
        im["x_bf"] = xT.astype(bf16)
        im["t_f32"] = tT
        im["t_bf"] = tT.astype(bf16)
        in_maps.append(im)
    return in_maps, bf2


def kernel(**inputs):
    with_bias = not all(
        np.allclose(np.asarray(inputs[k]), 0.0)
        for k in ("b1", "bproj", "b2", "bf1"))
    key = ("nc", with_bias)
    if key not in _CACHE:
        _CACHE[key] = _build_program(with_bias)
    nc = _CACHE[key]
    in_maps, bf2 = _prep_host(inputs)
    res = run_bass_kernel_spmd(nc, in_maps, core_ids=list(range(N_CORES)),
                               **_RUN_KWARGS)
    _LAST_RES[0] = res
    out = np.zeros((B, S, D), np.float32)
    out_tan = np.zeros((B, M, S, D), np.float32)
    for core in range(N_CORES):
        b, mp = core // 2, core % 2
        o = res.results[core]["out"].reshape(3, D, S)
        if mp == 0:
            out[b] = o[0].T + bf2[None, :]
        out_tan[b, 2 * mp] = o[1].T
        out_tan[b, 2 * mp + 1] = o[2].T
    return out, out_tan
